# revision 1
# baseline (speedup 1.0000x reference)
"""GAT layer on 8 Trainium2 NeuronCores (Bass/Tile), edge-parallel dst-sharded.

Self-contained. Host preprocesses the graph (dst-shard, bucket sort with
uniform caps, A/B split of each bucket's edges by source half for int16
gather indices; self-loops are NOT materialized as edges). Device program:

  phase 1: per 128-node tile, matmul x @ [W.T | a-folded] producing rows
    [Wh1(264) | el(8) | er(8)]; Wh1 interleaves a constant 1.0 after each
    head's 32 channels so a single multiply by w=exp(leaky(e)) yields both
    the weighted message AND the softmax-denominator column. Rows go to two
    local half-tables (stride 384 for dma_gather's 256B-stride rule) and a
    compact local er table.
  AllGather x2: half tables -> tblA/tblB on every core (AG1 fires once the
    first 25 buckets are written and overlaps the rest of phase 1).
  er expansion on the TENSOR engine (no DMA gather): per bucket, a
    transposed one-hot OTT (built from host-shipped per-slot dst indices)
    times the bucket's er rows gives per-edge-slot er for every block -
    packed into one PSUM tile per bucket and copied out. Runs during the
    AllGather window.
  phase 2 per super-chunk: two dma_gather calls (A/B) fetch table rows by
    src; score ops (add on DVE, leaky+exp on the scalar engine); one-op
    one-hot build; per-block V multiplies; per bucket capA+capB one-hot
    scatter matmuls accumulated in PSUM, then a final identity-matmul adds
    the analytically-computed self-loop contribution, and the bucket is
    normalized by the gathered denominator column and written out.

The SWDGE (GpSimd) descriptor path only carries the unavoidable per-edge
table gather; everything index-like that is bucket-local (er by dst, the
self loops) rides the tensor engine instead.
"""
import sys

for _p in ("/opt/trn_rl_repo",):
    if _p not in sys.path:
        sys.path.insert(0, _p)

import numpy as np
import ml_dtypes

import concourse.bass as bass
import concourse.tile as tile
from concourse import mybir, library_config
from concourse.bass_utils import run_bass_kernel_spmd
from concourse.ap_utils import ap_is_contiguous
from concourse.library_overlay import lower_extended_insts

BF16 = ml_dtypes.bfloat16

N = 50000
E = 800000
IN = 256
H = 8
C = 32
C1 = C + 1            # 33: [Wh_h(32) | 1]
HC = H * C            # 256
WH1 = H * C1          # 264
NC = 8
NPC = N // NC         # 6250 nodes per core
BUCKET = 128
NBUCK = (NPC + BUCKET - 1) // BUCKET   # 49
XT_PAD = NBUCK * 128                   # 6272
PAY = WH1 + H         # 272: gather payload [Wh1(264) | el(8)]
P1COLS = PAY + H      # 280: phase-1 matmul out [Wh1 | el | er]
TROW = 384            # table row stride (256B multiple)
GELEM = 272           # gathered elements per row
NEG = 0.2
EPS = 1e-16
SC_BUCKETS = 2        # buckets per gather super-chunk
NBUCK_A = 25          # buckets 0..24 -> table half A
ROWS_A = NBUCK_A * BUCKET          # 3200
ROWS_B = NPC - ROWS_A              # 3050
USE_PREP_TRIGGER = False
USE_ACT_LRELU = False

_waitfix_ctr = [0]
_reg_cache = {}


def _split_excess_waits(nc, max_waits=1):
    # walrus in this container caps sync waits per instruction at 1; hoist
    # excess onto same-engine NoOps.
    n_fixed = 0
    for fn in nc.m.functions:
        for bb in fn.blocks:
            insts = bb.instructions
            out = []
            for ins in insts:
                si = ins.sync_info
                waits = list(si.on_wait) if si is not None and si.on_wait else []
                if len(waits) > max_waits:
                    keep = waits[-max_waits:]
                    extra = waits[:-max_waits]
                    for i in range(0, len(extra), max_waits):
                        grp = extra[i:i + max_waits]
                        _waitfix_ctr[0] += 1
                        nop = mybir.InstNoOp(
                            name=f"I-waitfix-{_waitfix_ctr[0]}", ins=[], outs=[])
                        nop.engine = ins.engine
                        nop.sync_info = mybir.SyncInfo(on_wait=grp, on_update=[])
                        nc.register_instruction(nop)
                        out.append(nop)
                    si.on_wait = keep
                    n_fixed += 1
                out.append(ins)
            if len(out) != len(insts):
                bb.instructions = out
    return n_fixed


def _move_reload_after_collectives(nc):
    """The tile scheduler floats the dependency-less library-reload pseudo to
    the top of the program; keep it after the last collective trigger."""
    from concourse import bass_isa
    for fn in nc.m.functions:
        for bb in fn.blocks:
            insts = bb.instructions
            reload_idx = [i for i, ins in enumerate(insts)
                          if isinstance(ins, bass_isa.InstPseudoReloadLibraryIndex)]
            coll_idx = [i for i, ins in enumerate(insts)
                        if isinstance(ins, mybir.InstCollectiveCompute)]
            if not reload_idx or not coll_idx:
                continue
            assert len(reload_idx) == 1
            r = reload_idx[0]
            last_c = max(coll_idx)
            if r > last_c:
                continue
            ins = insts.pop(r)
            insts.insert(last_c, ins)
            bb.instructions = insts
    return nc


def _dma_gather_raw(eng, out_ap, in_ap, idxs_ap, num_idxs, elem_size, elem_step,
                    sem=None):
    """bass.dma_gather without the elem_size_bytes%256 assert (non-transpose,
    DRAM source, 256B-multiple row stride). sem!=None -> prepare_only."""
    assert idxs_ap.dtype == mybir.dt.int16
    assert in_ap.dtype == out_ap.dtype
    assert ap_is_contiguous(out_ap.ap[1:])
    assert ap_is_contiguous(idxs_ap.ap[1:])
    assert in_ap.ap[0][0] == elem_step
    stride_bytes = elem_step * mybir.dt.size(in_ap.dtype)
    stride_bytes_256 = stride_bytes // 256
    assert stride_bytes_256 * 256 == stride_bytes and stride_bytes_256 < 256
    _in_ap = eng.lower_ap_dma(in_ap, for_custom_bir_dma=True)
    _idxs_ap = eng.lower_ap(idxs_ap)
    _out_ap = eng.lower_ap(out_ap)
    key = (id(eng.bass), num_idxs)
    if key not in _reg_cache:
        _reg_cache[key] = eng.to_reg(num_idxs)
    inst = eng.add_instruction(
        mybir.InstDMAGatherAnt(
            name=eng.bass.get_next_instruction_name(),
            ins=[*_in_ap, _idxs_ap, eng.lower_val_access(_reg_cache[key])],
            outs=[_out_ap],
            transpose=False,
            num_idxs=num_idxs,
            elem_size=elem_size,
            stride_bytes_256=stride_bytes_256,
            gen_mode=int(sem is not None),
            single_packet=False,
            queue_num=0,
            sbuf_tokens_per_rank=0,
            sbuf_free_dim_per_rank=0,
            sbuf_free_dim_pad_per_rank=0,
            sbuf_byte_offset=0,
        )
    )
    if sem is not None:
        inst.then_inc(sem, 16)
        return eng._track_prepare_only(inst, 0)
    return inst


def _wrap16(vals):
    """Edge-slot int16 index array -> dma_gather layout [128, n/16]."""
    n = len(vals)
    assert n % 16 == 0
    w = np.asarray(vals, np.int16).reshape(n // 16, 16).T
    return np.tile(w, (8, 1))


def _host_prep(x, edge_index, W, a_left, a_right):
    src = np.asarray(edge_index[0], np.int64)
    dst = np.asarray(edge_index[1], np.int64)

    # fold attention vectors through W:  [el|er] = x @ (W.T @ A)
    A = np.zeros((HC, 2 * H), np.float32)
    for h in range(H):
        A[h * C:(h + 1) * C, h] = a_left[h]
        A[h * C:(h + 1) * C, H + h] = a_right[h]
    B = (W.T.astype(np.float64) @ A.astype(np.float64)).astype(np.float32)
    wtb = np.zeros((IN, P1COLS), np.float32)
    for h in range(H):
        wtb[:, h * C1:h * C1 + C] = W.T[:, h * C:(h + 1) * C]
    wtb[:, WH1:WH1 + H] = B[:, :H]
    wtb[:, WH1 + H:] = B[:, H:]
    wtb = wtb.astype(BF16)

    core = dst // NPC
    r_src = src % NPC
    c_src = src // NPC
    is_a = r_src < ROWS_A
    gidx = np.where(is_a, c_src * ROWS_A + r_src,
                    c_src * ROWS_B + (r_src - ROWS_A)).astype(np.int64)

    capA = capB = 0
    lists = {}
    for c in range(NC):
        m = core == c
        s_c, d_c, g_c, a_c = src[m], dst[m], gidx[m], is_a[m]
        dl = d_c - c * NPC
        b_c = dl // BUCKET
        order = np.lexsort((s_c, b_c))
        s_c, dl, b_c, g_c, a_c = (s_c[order], dl[order], b_c[order],
                                  g_c[order], a_c[order])
        cnt = np.bincount(b_c, minlength=NBUCK)
        starts = np.concatenate([[0], np.cumsum(cnt)[:-1]])
        for b in range(NBUCK):
            sl = slice(starts[b], starts[b] + cnt[b])
            aa = a_c[sl]
            lists[(c, b)] = (g_c[sl][aa], dl[sl][aa] - b * BUCKET,
                             g_c[sl][~aa], dl[sl][~aa] - b * BUCKET)
            capA = max(capA, (int(aa.sum()) + 127) // 128)
            capB = max(capB, (int((~aa).sum()) + 127) // 128)

    nblkb = capA + capB
    nblk = NBUCK * nblkb
    nblkA = NBUCK * capA
    nblkB = NBUCK * capB
    n_sc = (NBUCK + SC_BUCKETS - 1) // SC_BUCKETS

    idxA = np.zeros((NC, 128, nblkA * 8), np.int16)
    idxB = np.zeros((NC, 128, nblkB * 8), np.int16)
    dloc_u = np.full((NC, 128, nblk), 200.0, BF16)
    dlocT = np.full((NC, 128, nblk * 128), -1, np.int8)
    xT = np.zeros((NC, IN, XT_PAD), BF16)

    for c in range(NC):
        iA = np.zeros(nblkA * 128, np.int64)
        iB = np.zeros(nblkB * 128, np.int64)
        dA = np.full((nblkA, 128), -1, np.int64)
        dB = np.full((nblkB, 128), -1, np.int64)
        for b in range(NBUCK):
            gA, dlA, gB, dlB = lists[(c, b)]
            oa = b * capA * 128
            ob = b * capB * 128
            iA[oa:oa + len(gA)] = gA
            iB[ob:ob + len(gB)] = gB
            fa = dA[b * capA:(b + 1) * capA].reshape(-1)
            fa[:len(dlA)] = dlA
            fb = dB[b * capB:(b + 1) * capB].reshape(-1)
            fb[:len(dlB)] = dlB
        idxA[c] = _wrap16(iA)
        idxB[c] = _wrap16(iB)
        # slot k of block j = (partition k%128); dA rows are flat slot runs
        dA = dA.reshape(nblkA, 128)
        dB = dB.reshape(nblkB, 128)
        # dloc_u: SC-major order [per SC: A-blocks | B-blocks], [128, nblk]
        off = 0
        for sc in range(n_sc):
            b0 = sc * SC_BUCKETS
            bs = range(b0, min(b0 + SC_BUCKETS, NBUCK))
            for b in bs:
                blk = dA[b * capA:(b + 1) * capA]     # [capA, 128]
                v = np.where(blk < 0, 200.0, blk).astype(np.float32)
                dloc_u[c, :, off:off + capA] = v.T.astype(BF16)
                off += capA
            for b in bs:
                blk = dB[b * capB:(b + 1) * capB]
                v = np.where(blk < 0, 200.0, blk).astype(np.float32)
                dloc_u[c, :, off:off + capB] = v.T.astype(BF16)
                off += capB
        # dlocT: BUCKET-major order [per bucket: A-blocks | B-blocks],
        # transposed and replicated: [128(any), (b*nblkb + j)*128 + p]
        for b in range(NBUCK):
            base = b * nblkb * 128
            rows = np.concatenate(
                [dA[b * capA:(b + 1) * capA], dB[b * capB:(b + 1) * capB]],
                axis=0)                                # [nblkb, 128]
            dlocT[c, :, base:base + nblkb * 128] = np.broadcast_to(
                rows.reshape(-1).astype(np.int8), (128, nblkb * 128))

        xs = x[c * NPC:(c + 1) * NPC].astype(BF16)
        xT[c, :, :NPC] = xs.T

    iota = np.tile(np.arange(128, dtype=np.float32)[None, :], (128, 1)).astype(BF16)
    iotaP = np.arange(128, dtype=np.int8).reshape(128, 1)
    iotaPb = np.arange(128, dtype=np.float32).reshape(128, 1).astype(BF16)
    return (wtb, idxA, idxB, dloc_u, dlocT, xT, iota, iotaP, iotaPb,
            capA, capB)


def _build_program(capA, capB):
    nblkb = capA + capB
    nblk = NBUCK * nblkb
    nblkA = NBUCK * capA
    nblkB = NBUCK * capB
    f32 = mybir.dt.float32
    bf16 = mybir.dt.bfloat16
    i16 = mybir.dt.int16
    i8 = mybir.dt.int8

    nc = bass.Bass(trn_type="TRN2", num_devices=NC)
    xT_in = nc.declare_dram_parameter("xT", [IN, XT_PAD], bf16, isOutput=False)
    wtb_in = nc.declare_dram_parameter("wtb", [IN, P1COLS], bf16, isOutput=False)
    idxA_in = nc.declare_dram_parameter("idxA", [128, nblkA * 8], i16, isOutput=False)
    idxB_in = nc.declare_dram_parameter("idxB", [128, nblkB * 8], i16, isOutput=False)
    dloc_in = nc.declare_dram_parameter("dloc", [128, nblk], bf16, isOutput=False)
    dlocT_in = nc.declare_dram_parameter("dlocT", [128, nblk * 128], i8, isOutput=False)
    iota_in = nc.declare_dram_parameter("iota", [128, 128], bf16, isOutput=False)
    iotaP_in = nc.declare_dram_parameter("iotaP", [128, 1], i8, isOutput=False)
    iotaPb_in = nc.declare_dram_parameter("iotaPb", [128, 1], bf16, isOutput=False)
    out_ext = nc.declare_dram_parameter("out", [NPC, HC], f32, isOutput=True)

    tbl_locA = nc.dram_tensor("tbl_locA", [ROWS_A, TROW], bf16)
    tbl_locB = nc.dram_tensor("tbl_locB", [ROWS_B, TROW], bf16)
    tblA = nc.dram_tensor("tblA", [NC * ROWS_A, TROW], bf16, addr_space="Shared")
    tblB = nc.dram_tensor("tblB", [NC * ROWS_B, TROW], bf16, addr_space="Shared")
    er_tbl = nc.dram_tensor("er_tbl", [NBUCK * 128, H], bf16)

    with tile.TileContext(nc) as tc:
        with tc.tile_pool(name="cst", bufs=1) as cst:
            # ---------------- phase 1: Wh1 / el / er ----------------
            with tc.tile_pool(name="p1w", bufs=1) as p1w, \
                 tc.tile_pool(name="p1", bufs=3) as p1, \
                 tc.tile_pool(name="ps1", bufs=2, space="PSUM") as ps1:
                xts = []
                wtbs = []
                for k in range(2):
                    t = p1w.tile([128, XT_PAD], bf16, tag=f"xt{k}")
                    nc.sync.dma_start(out=t[:], in_=xT_in[k * 128:(k + 1) * 128, :])
                    xts.append(t)
                    u = p1w.tile([128, P1COLS], bf16, tag=f"wtb{k}")
                    nc.sync.dma_start(out=u[:], in_=wtb_in[k * 128:(k + 1) * 128, :])
                    wtbs.append(u)
                for tn in range(NBUCK):
                    ps = ps1.tile([128, P1COLS], f32)
                    for k in range(2):
                        nc.tensor.matmul(
                            out=ps[:],
                            lhsT=xts[k][:, tn * 128:(tn + 1) * 128],
                            rhs=wtbs[k][:],
                            start=(k == 0), stop=(k == 1),
                        )
                    sb = p1.tile([128, P1COLS], bf16)
                    nc.scalar.activation(out=sb[:], in_=ps[:],
                                         func=mybir.ActivationFunctionType.Copy)
                    sb4 = sb[:, 0:WH1].rearrange("p (h c) -> p h c", c=C1)
                    nc.vector.memset(sb4[:, :, C:C1], 1.0)
                    rows = min(128, NPC - tn * 128)
                    if tn < NBUCK_A:
                        trow = tbl_locA[tn * 128:tn * 128 + rows, 0:PAY]
                    else:
                        r0 = tn * 128 - ROWS_A
                        trow = tbl_locB[r0:r0 + rows, 0:PAY]
                    nc.sync.dma_start(out=trow, in_=sb[:rows, 0:PAY])
                    nc.sync.dma_start(
                        out=er_tbl[tn * 128:tn * 128 + rows, :],
                        in_=sb[:rows, PAY:P1COLS])

            # ---------------- all-gather both half tables ----------------
            nc.gpsimd.collective_compute(
                "AllGather", mybir.AluOpType.bypass,
                replica_groups=[list(range(NC))],
                ins=[tbl_locA[:].opt()], outs=[tblA[:].opt()],
            )
            nc.gpsimd.collective_compute(
                "AllGather", mybir.AluOpType.bypass,
                replica_groups=[list(range(NC))],
                ins=[tbl_locB[:].opt()], outs=[tblB[:].opt()],
            )
            nc.gpsimd.load_library(library_config.mlp)

            iota_b = cst.tile([128, 128], bf16)
            nc.sync.dma_start(out=iota_b[:], in_=iota_in[:, :])
            iota_m = iota_b[:].rearrange("p (b n) -> p b n", b=1)
            iotaP_sb = cst.tile([128, 1], i8)
            nc.sync.dma_start(out=iotaP_sb[:], in_=iotaP_in[:, :])
            iotaP_m = iotaP_sb[:].rearrange("p (b n) -> p b n", b=1)
            iotaPb_sb = cst.tile([128, 1], bf16)
            nc.sync.dma_start(out=iotaPb_sb[:], in_=iotaPb_in[:, :])
            ident = cst.tile([128, 128], bf16)
            nc.vector.tensor_tensor(
                out=ident[:], in0=iotaPb_sb[:].to_broadcast([128, 128]),
                in1=iota_b[:], op=mybir.AluOpType.is_equal)

            idxA_sb = cst.tile([128, nblkA * 8], i16)
            nc.sync.dma_start(out=idxA_sb[:], in_=idxA_in[:, :])
            idxB_sb = cst.tile([128, nblkB * 8], i16)
            nc.sync.dma_start(out=idxB_sb[:], in_=idxB_in[:, :])
            dloc_sb = cst.tile([128, nblk], bf16)
            nc.sync.dma_start(out=dloc_sb[:], in_=dloc_in[:, :])
            # er for all buckets, bucket-partition layout: [p, b, h]
            er_full = cst.tile([128, NBUCK * H], bf16)
            er_full3 = er_full[:].rearrange("p (b h) -> p b h", h=H)
            er_in3 = er_tbl[:(NBUCK - 1) * 128].rearrange("(b p) h -> p b h", p=128)
            nc.sync.dma_start(out=er_full3[:, 0:NBUCK - 1, :], in_=er_in3[:, :, :])
            # last bucket is partial (106 rows): zero first so the OTT matmul
            # can't pick up non-finite garbage from the dead partitions
            nc.vector.memset(er_full3[:, NBUCK - 1, :], 0.0)
            lastr = NPC - (NBUCK - 1) * 128
            nc.sync.dma_start(
                out=er_full3[0:lastr, NBUCK - 1, :],
                in_=er_tbl[(NBUCK - 1) * 128:NPC, :])

            # ---------------- er expansion on PE (overlaps AllGather) ------
            er_e = cst.tile([128, nblk * 8], bf16)   # SC-major slot order
            with tc.tile_pool(name="otq", bufs=2) as otq, \
                 tc.tile_pool(name="dtq", bufs=2) as dtq, \
                 tc.tile_pool(name="pse", bufs=2, space="PSUM") as pse:
                n_sc = (NBUCK + SC_BUCKETS - 1) // SC_BUCKETS
                for b in range(NBUCK):
                    dT = dtq.tile([128, nblkb * 128], i8, tag="dT")
                    nc.sync.dma_start(
                        out=dT[:],
                        in_=dlocT_in[:, b * nblkb * 128:(b + 1) * nblkb * 128])
                    dT3 = dT[:].rearrange("p (j n) -> p j n", n=128)
                    OTT = otq.tile([128, nblkb * 128], bf16, tag="OTT")
                    OTT3 = OTT[:].rearrange("p (j n) -> p j n", n=128)
                    nc.vector.tensor_tensor(
                        out=OTT3, in0=iotaP_m.to_broadcast([128, nblkb, 128]),
                        in1=dT3, op=mybir.AluOpType.is_equal)
                    pe = pse.tile([128, nblkb * H], f32)
                    pe3 = pe[:].rearrange("p (j h) -> p j h", h=H)
                    for j in range(nblkb):
                        nc.tensor.matmul(
                            out=pe3[:, j, :], lhsT=OTT3[:, j, :],
                            rhs=er_full3[:, b, :], start=True, stop=True)
                    # copy into er_e at SC-major positions (A-run | B-run)
                    sc = b // SC_BUCKETS
                    b0 = sc * SC_BUCKETS
                    nbk = min(SC_BUCKETS, NBUCK - b0)
                    off_sc = b0 * nblkb
                    offA = (off_sc + (b - b0) * capA) * 8
                    offB = (off_sc + nbk * capA + (b - b0) * capB) * 8
                    nc.vector.tensor_copy(
                        out=er_e[:, offA:offA + capA * 8],
                        in_=pe[:, 0:capA * 8])
                    nc.vector.tensor_copy(
                        out=er_e[:, offB:offB + capB * 8],
                        in_=pe[:, capA * 8:nblkb * 8])

            # ---------------- phase 2: gather / score / scatter ------------
            with tc.tile_pool(name="gp", bufs=4) as gp, \
                 tc.tile_pool(name="vp", bufs=2) as vp, \
                 tc.tile_pool(name="otp", bufs=2) as otp, \
                 tc.tile_pool(name="sp", bufs=2) as sp, \
                 tc.tile_pool(name="sf", bufs=3) as sf, \
                 tc.tile_pool(name="np_", bufs=3) as np_, \
                 tc.tile_pool(name="ps2", bufs=4, space="PSUM") as ps2p:

                n_sc = (NBUCK + SC_BUCKETS - 1) // SC_BUCKETS
                pend = {}

                def emit_A(sc):
                    b0 = sc * SC_BUCKETS
                    nbk = min(SC_BUCKETS, NBUCK - b0)
                    nbA = nbk * capA
                    nbB = nbk * capB
                    nb = nbA + nbB
                    G = gp.tile([128, nb * GELEM], bf16, tag="G")
                    G3 = G[:].rearrange("p (b y) -> p b y", y=GELEM)
                    a0 = b0 * capA * 8
                    _dma_gather_raw(
                        nc.gpsimd, G3[:, 0:nbA, :], tblA[:],
                        idxA_sb[:, a0:a0 + nbA * 8], nbA * 128,
                        GELEM, TROW)
                    pend[sc] = (G3, b0, nbk, nbA, nbB, nb)

                def emit_B(sc):
                    G3, b0, nbk, nbA, nbB, nb = pend[sc]
                    bb0 = b0 * capB * 8
                    _dma_gather_raw(
                        nc.gpsimd, G3[:, nbA:nb, :], tblB[:],
                        idxB_sb[:, bb0:bb0 + nbB * 8], nbB * 128,
                        GELEM, TROW)

                def emit_compute(sc):
                    G3, b0, nbk, nbA, nbB, nb = pend.pop(sc)
                    off_u = b0 * nblkb
                    # scores: e = el + er ; leaky ; exp
                    e_t = sp.tile([128, nb * H], f32, tag="e")
                    e3 = e_t[:].rearrange("p (b h) -> p b h", h=H)
                    er_sc3 = er_e[:, off_u * 8:(off_u + nb) * 8].rearrange(
                        "p (b h) -> p b h", h=H)
                    nc.vector.tensor_tensor(
                        out=e3, in0=G3[:, :, WH1:PAY], in1=er_sc3,
                        op=mybir.AluOpType.add)
                    # w = exp(leaky(e)) = max(exp(e), exp(NEG*e)) (exp monotone)
                    w_t = sp.tile([128, nb * H], bf16, tag="w")
                    w1_t = sp.tile([128, nb * H], bf16, tag="w1")
                    nc.scalar.activation(
                        out=w1_t[:], in_=e_t[:],
                        func=mybir.ActivationFunctionType.Exp)
                    w2_t = sp.tile([128, nb * H], bf16, tag="w2")
                    nc.scalar.activation(
                        out=w2_t[:], in_=e_t[:], scale=NEG,
                        func=mybir.ActivationFunctionType.Exp)
                    nc.vector.tensor_tensor(
                        out=w_t[:], in0=w1_t[:], in1=w2_t[:],
                        op=mybir.AluOpType.max)
                    w3 = w_t[:].rearrange("p (b h) -> p b h", h=H)

                    # one-op one-hot build for the whole super-chunk
                    OT = otp.tile([128, nb * 128], bf16, tag="OT")
                    OT3 = OT[:].rearrange("p (b n) -> p b n", n=128)
                    d3 = dloc_sb[:, off_u:off_u + nb].to_broadcast([128, nb, 128])
                    i3 = iota_m.to_broadcast([128, nb, 128])
                    nc.vector.tensor_tensor(out=OT3, in0=d3, in1=i3,
                                            op=mybir.AluOpType.is_equal)

                    # per-block V multiplies
                    V = vp.tile([128, nb * WH1], bf16, tag="V")
                    V3 = V[:].rearrange("p (b y) -> p b y", y=WH1)
                    for blk in range(nb):
                        V4b = V3[:, blk, :].rearrange("p (h c) -> p h c", c=C1)
                        G4b = G3[:, blk, 0:WH1].rearrange("p (h c) -> p h c", c=C1)
                        w4b = w3[:, blk, :].to_broadcast([128, H, C1])
                        nc.vector.tensor_tensor(out=V4b, in0=G4b, in1=w4b,
                                                op=mybir.AluOpType.mult)

                    # per bucket: scatter matmuls + self-loop fold + normalize
                    for bb in range(nbk):
                        bucket = b0 + bb
                        # self-loop contribution
                        ts = sf.tile([128, PAY], bf16, tag="ts")
                        if bucket < NBUCK_A:
                            tsrc = tbl_locA[bucket * 128:bucket * 128 + 128, 0:PAY]
                        else:
                            r0 = bucket * 128 - ROWS_A
                            rows = min(128, ROWS_B - r0)
                            tsrc = tbl_locB[r0:r0 + rows, 0:PAY]
                        nc.sync.dma_start(out=ts[:tsrc.shape[0], :], in_=tsrc)
                        es_s = sf.tile([128, H], f32, tag="es_s")
                        nc.vector.tensor_tensor(
                            out=es_s[:], in0=ts[:, WH1:PAY],
                            in1=er_full3[:, bucket, :], op=mybir.AluOpType.add)
                        ws_s = sf.tile([128, H], bf16, tag="ws_s")
                        ws1 = sf.tile([128, H], bf16, tag="ws1")
                        nc.scalar.activation(
                            out=ws1[:], in_=es_s[:],
                            func=mybir.ActivationFunctionType.Exp)
                        ws2 = sf.tile([128, H], bf16, tag="ws2")
                        nc.scalar.activation(
                            out=ws2[:], in_=es_s[:], scale=NEG,
                            func=mybir.ActivationFunctionType.Exp)
                        nc.vector.tensor_tensor(
                            out=ws_s[:], in0=ws1[:], in1=ws2[:],
                            op=mybir.AluOpType.max)
                        vs = sf.tile([128, WH1], bf16, tag="vs")
                        vs4 = vs[:].rearrange("p (h c) -> p h c", c=C1)
                        ts4 = ts[:, 0:WH1].rearrange("p (h c) -> p h c", c=C1)
                        nc.vector.tensor_tensor(
                            out=vs4, in0=ts4,
                            in1=ws_s[:].to_broadcast([128, H, C1]),
                            op=mybir.AluOpType.mult)

                        blks = ([bb * capA + j for j in range(capA)] +
                                [nbA + bb * capB + j for j in range(capB)])
                        ps = ps2p.tile([128, WH1], f32)
                        for i, blk in enumerate(blks):
                            nc.tensor.matmul(
                                out=ps[:], lhsT=OT3[:, blk, :], rhs=V3[:, blk, :],
                                start=(i == 0), stop=False)
                        nc.tensor.matmul(
                            out=ps[:], lhsT=ident[:], rhs=vs[:],
                            start=False, stop=True)

                        ps4 = ps[:].rearrange("p (h c) -> p h c", c=C1)
                        den = np_.tile([128, H], f32, tag="den")
                        nc.vector.tensor_scalar_add(den[:], ps4[:, :, C], EPS)
                        rec = np_.tile([128, H], f32, tag="rec")
                        nc.vector.reciprocal(rec[:], den[:])
                        ot = np_.tile([128, HC], f32, tag="ot")
                        ot3 = ot[:].rearrange("p (h c) -> p h c", c=C)
                        r3 = rec[:].to_broadcast([128, H, C])
                        nc.vector.tensor_tensor(out=ot3, in0=ps4[:, :, 0:C],
                                                in1=r3, op=mybir.AluOpType.mult)
                        rows = min(128, NPC - bucket * 128)
                        nc.sync.dma_start(
                            out=out_ext[bucket * 128:bucket * 128 + rows, :],
                            in_=ot[:rows, :])

                LAG = 3
                for sc in range(n_sc):
                    emit_A(sc)
                    if sc >= LAG:
                        emit_B(sc - LAG)
                        emit_compute(sc - LAG)
                for sc in range(max(0, n_sc - LAG), n_sc):
                    emit_B(sc)
                    emit_compute(sc)

    _split_excess_waits(nc)
    _move_reload_after_collectives(nc)
    lower_extended_insts(nc)
    return nc


def kernel(**inputs):
    x = np.asarray(inputs["x"], np.float32)
    edge_index = np.asarray(inputs["edge_index"])
    W = np.asarray(inputs["W"], np.float32)
    a_left = np.asarray(inputs["a_left"], np.float32)
    a_right = np.asarray(inputs["a_right"], np.float32)

    (wtb, idxA, idxB, dloc_u, dlocT, xT, iota, iotaP, iotaPb,
     capA, capB) = _host_prep(x, edge_index, W, a_left, a_right)
    nc = _build_program(capA, capB)

    in_maps = []
    for c in range(NC):
        in_maps.append({
            "xT": np.ascontiguousarray(xT[c]),
            "wtb": wtb,
            "idxA": np.ascontiguousarray(idxA[c]),
            "idxB": np.ascontiguousarray(idxB[c]),
            "dloc": np.ascontiguousarray(dloc_u[c]),
            "dlocT": np.ascontiguousarray(dlocT[c]),
            "iota": iota,
            "iotaP": iotaP,
            "iotaPb": iotaPb,
        })

    res = run_bass_kernel_spmd(nc, in_maps, core_ids=list(range(NC)))
    out = np.concatenate([np.asarray(res.results[c]["out"]) for c in range(NC)], axis=0)
    return out.astype(np.float32)



# revision 4
# speedup vs baseline: 1.3983x; 1.3983x over previous
"""GAT layer on 8 Trainium2 NeuronCores (Bass/Tile), edge-parallel dst-sharded.

Self-contained. Host preprocesses the graph (dst-shard, bucket sort with
uniform caps, A/B split of each bucket's edges by source half for int16
gather indices; self-loops are NOT materialized as edges). Device program:

  phase 1: per 128-node tile, matmul x @ [W.T | a-folded] producing rows
    [Wh1(264) | el(8) | er(8)]; Wh1 interleaves a constant 1.0 after each
    head's 32 channels so a single multiply by w=exp(leaky(e)) yields both
    the weighted message AND the softmax-denominator column. Rows go to two
    local half-tables (stride 384 for dma_gather's 256B-stride rule) and a
    compact local er table.
  AllGather x2: half tables -> tblA/tblB on every core (AG1 fires once the
    first 25 buckets are written and overlaps the rest of phase 1).
  er expansion on the TENSOR engine (no DMA gather): per bucket, a
    transposed one-hot OTT (built from host-shipped per-slot dst indices)
    times the bucket's er rows gives per-edge-slot er for every block -
    packed into one PSUM tile per bucket and copied out. Runs during the
    AllGather window.
  phase 2 per super-chunk: two dma_gather calls (A/B) fetch table rows by
    src; score ops (add on DVE, leaky+exp on the scalar engine); one-op
    one-hot build; per-block V multiplies; per bucket capA+capB one-hot
    scatter matmuls accumulated in PSUM, then a final identity-matmul adds
    the analytically-computed self-loop contribution, and the bucket is
    normalized by the gathered denominator column and written out.

The SWDGE (GpSimd) descriptor path only carries the unavoidable per-edge
table gather; everything index-like that is bucket-local (er by dst, the
self loops) rides the tensor engine instead.
"""
import sys

for _p in ("/opt/trn_rl_repo",):
    if _p not in sys.path:
        sys.path.insert(0, _p)

import numpy as np
import ml_dtypes

import concourse.bass as bass
import concourse.tile as tile
from concourse import mybir, library_config
from concourse.bass_utils import run_bass_kernel_spmd
from concourse.ap_utils import ap_is_contiguous
from concourse.library_overlay import lower_extended_insts

BF16 = ml_dtypes.bfloat16

N = 50000
E = 800000
IN = 256
H = 8
C = 32
C1 = C + 1            # 33: [Wh_h(32) | 1]
HC = H * C            # 256
WH1 = H * C1          # 264
NC = 8
NPC = N // NC         # 6250 nodes per core
BUCKET = 128
NBUCK = (NPC + BUCKET - 1) // BUCKET   # 49
XT_PAD = NBUCK * 128                   # 6272
PAY = WH1 + H         # 272: gather payload [Wh1(264) | el(8)]
P1COLS = PAY + H      # 280: phase-1 matmul out [Wh1 | el | er]
TROW = 384            # table row stride (256B multiple)
GELEM = 272           # gathered elements per row
NEG = 0.2
EPS = 1e-16
SC_BUCKETS = 2        # buckets per gather super-chunk
NBUCK_A = 25          # buckets 0..24 -> table half A
ROWS_A = NBUCK_A * BUCKET          # 3200
ROWS_B = NPC - ROWS_A              # 3050
USE_PREP_TRIGGER = False
USE_ACT_LRELU = False
NQ = 4

_waitfix_ctr = [0]
_reg_cache = {}


def _split_excess_waits(nc, max_waits=1):
    # walrus in this container caps sync waits per instruction at 1; hoist
    # excess onto same-engine NoOps.
    n_fixed = 0
    for fn in nc.m.functions:
        for bb in fn.blocks:
            insts = bb.instructions
            out = []
            for ins in insts:
                si = ins.sync_info
                waits = list(si.on_wait) if si is not None and si.on_wait else []
                if len(waits) > max_waits:
                    keep = waits[-max_waits:]
                    extra = waits[:-max_waits]
                    for i in range(0, len(extra), max_waits):
                        grp = extra[i:i + max_waits]
                        _waitfix_ctr[0] += 1
                        nop = mybir.InstNoOp(
                            name=f"I-waitfix-{_waitfix_ctr[0]}", ins=[], outs=[])
                        nop.engine = ins.engine
                        nop.sync_info = mybir.SyncInfo(on_wait=grp, on_update=[])
                        nc.register_instruction(nop)
                        out.append(nop)
                    si.on_wait = keep
                    n_fixed += 1
                out.append(ins)
            if len(out) != len(insts):
                bb.instructions = out
    return n_fixed


def _move_reload_after_collectives(nc):
    """The tile scheduler floats the dependency-less library-reload pseudo to
    the top of the program; keep it after the last collective trigger."""
    from concourse import bass_isa
    for fn in nc.m.functions:
        for bb in fn.blocks:
            insts = bb.instructions
            reload_idx = [i for i, ins in enumerate(insts)
                          if isinstance(ins, bass_isa.InstPseudoReloadLibraryIndex)]
            coll_idx = [i for i, ins in enumerate(insts)
                        if isinstance(ins, mybir.InstCollectiveCompute)]
            if not reload_idx or not coll_idx:
                continue
            assert len(reload_idx) == 1
            r = reload_idx[0]
            last_c = max(coll_idx)
            if r > last_c:
                continue
            ins = insts.pop(r)
            insts.insert(last_c, ins)
            bb.instructions = insts
    return nc


def _dma_gather_raw(eng, out_ap, in_ap, idxs_ap, num_idxs, elem_size, elem_step,
                    sem=None, queue_num=0):
    """bass.dma_gather without the elem_size_bytes%256 assert (non-transpose,
    DRAM source, 256B-multiple row stride). sem!=None -> prepare_only."""
    assert idxs_ap.dtype == mybir.dt.int16
    assert in_ap.dtype == out_ap.dtype
    assert ap_is_contiguous(out_ap.ap[1:])
    assert ap_is_contiguous(idxs_ap.ap[1:])
    assert in_ap.ap[0][0] == elem_step
    stride_bytes = elem_step * mybir.dt.size(in_ap.dtype)
    stride_bytes_256 = stride_bytes // 256
    assert stride_bytes_256 * 256 == stride_bytes and stride_bytes_256 < 256
    _in_ap = eng.lower_ap_dma(in_ap, for_custom_bir_dma=True)
    _idxs_ap = eng.lower_ap(idxs_ap)
    _out_ap = eng.lower_ap(out_ap)
    key = (id(eng.bass), num_idxs)
    if key not in _reg_cache:
        _reg_cache[key] = eng.to_reg(num_idxs)
    inst = eng.add_instruction(
        mybir.InstDMAGatherAnt(
            name=eng.bass.get_next_instruction_name(),
            ins=[*_in_ap, _idxs_ap, eng.lower_val_access(_reg_cache[key])],
            outs=[_out_ap],
            transpose=False,
            num_idxs=num_idxs,
            elem_size=elem_size,
            stride_bytes_256=stride_bytes_256,
            gen_mode=int(sem is not None),
            single_packet=False,
            queue_num=queue_num,
            sbuf_tokens_per_rank=0,
            sbuf_free_dim_per_rank=0,
            sbuf_free_dim_pad_per_rank=0,
            sbuf_byte_offset=0,
        )
    )
    if sem is not None:
        inst.then_inc(sem, 16)
        return eng._track_prepare_only(inst, queue_num)
    return inst


def _wrap16(vals):
    """Edge-slot int16 index array -> dma_gather layout [128, n/16]."""
    n = len(vals)
    assert n % 16 == 0
    w = np.asarray(vals, np.int16).reshape(n // 16, 16).T
    return np.tile(w, (8, 1))


def _host_prep(x, edge_index, W, a_left, a_right):
    src = np.asarray(edge_index[0], np.int64)
    dst = np.asarray(edge_index[1], np.int64)

    # fold attention vectors through W:  [el|er] = x @ (W.T @ A)
    A = np.zeros((HC, 2 * H), np.float32)
    for h in range(H):
        A[h * C:(h + 1) * C, h] = a_left[h]
        A[h * C:(h + 1) * C, H + h] = a_right[h]
    B = (W.T.astype(np.float64) @ A.astype(np.float64)).astype(np.float32)
    wtb = np.zeros((IN, P1COLS), np.float32)
    for h in range(H):
        wtb[:, h * C1:h * C1 + C] = W.T[:, h * C:(h + 1) * C]
    wtb[:, WH1:WH1 + H] = B[:, :H]
    wtb[:, WH1 + H:] = B[:, H:]
    wtb = wtb.astype(BF16)

    core = dst // NPC
    r_src = src % NPC
    c_src = src // NPC
    is_a = r_src < ROWS_A
    gidx = np.where(is_a, c_src * ROWS_A + r_src,
                    c_src * ROWS_B + (r_src - ROWS_A)).astype(np.int64)

    capA = capB = 0
    lists = {}
    for c in range(NC):
        m = core == c
        s_c, d_c, g_c, a_c = src[m], dst[m], gidx[m], is_a[m]
        dl = d_c - c * NPC
        b_c = dl // BUCKET
        order = np.lexsort((s_c, b_c))
        s_c, dl, b_c, g_c, a_c = (s_c[order], dl[order], b_c[order],
                                  g_c[order], a_c[order])
        cnt = np.bincount(b_c, minlength=NBUCK)
        starts = np.concatenate([[0], np.cumsum(cnt)[:-1]])
        for b in range(NBUCK):
            sl = slice(starts[b], starts[b] + cnt[b])
            aa = a_c[sl]
            lists[(c, b)] = (g_c[sl][aa], dl[sl][aa] - b * BUCKET,
                             g_c[sl][~aa], dl[sl][~aa] - b * BUCKET)
            capA = max(capA, (int(aa.sum()) + 127) // 128)
            capB = max(capB, (int((~aa).sum()) + 127) // 128)

    nblkb = capA + capB
    nblk = NBUCK * nblkb
    nblkA = NBUCK * capA
    nblkB = NBUCK * capB
    n_sc = (NBUCK + SC_BUCKETS - 1) // SC_BUCKETS

    idxA = np.zeros((NC, 128, nblkA * 8), np.int16)
    idxB = np.zeros((NC, 128, nblkB * 8), np.int16)
    dloc_u = np.full((NC, 128, nblk), 200.0, BF16)
    dlocT = np.full((NC, 128, nblk * 128), -1, np.int8)
    xT = np.zeros((NC, IN, XT_PAD), BF16)

    for c in range(NC):
        iA = np.zeros(nblkA * 128, np.int64)
        iB = np.zeros(nblkB * 128, np.int64)
        dA = np.full((nblkA, 128), -1, np.int64)
        dB = np.full((nblkB, 128), -1, np.int64)
        for b in range(NBUCK):
            gA, dlA, gB, dlB = lists[(c, b)]
            oa = b * capA * 128
            ob = b * capB * 128
            iA[oa:oa + len(gA)] = gA
            iB[ob:ob + len(gB)] = gB
            fa = dA[b * capA:(b + 1) * capA].reshape(-1)
            fa[:len(dlA)] = dlA
            fb = dB[b * capB:(b + 1) * capB].reshape(-1)
            fb[:len(dlB)] = dlB
        idxA[c] = _wrap16(iA)
        idxB[c] = _wrap16(iB)
        # slot k of block j = (partition k%128); dA rows are flat slot runs
        dA = dA.reshape(nblkA, 128)
        dB = dB.reshape(nblkB, 128)
        # dloc_u: SC-major order [per SC: A-blocks | B-blocks], [128, nblk]
        off = 0
        for sc in range(n_sc):
            b0 = sc * SC_BUCKETS
            bs = range(b0, min(b0 + SC_BUCKETS, NBUCK))
            for b in bs:
                blk = dA[b * capA:(b + 1) * capA]     # [capA, 128]
                v = np.where(blk < 0, 200.0, blk).astype(np.float32)
                dloc_u[c, :, off:off + capA] = v.T.astype(BF16)
                off += capA
            for b in bs:
                blk = dB[b * capB:(b + 1) * capB]
                v = np.where(blk < 0, 200.0, blk).astype(np.float32)
                dloc_u[c, :, off:off + capB] = v.T.astype(BF16)
                off += capB
        # dlocT: BUCKET-major order [per bucket: A-blocks | B-blocks],
        # transposed and replicated: [128(any), (b*nblkb + j)*128 + p]
        for b in range(NBUCK):
            base = b * nblkb * 128
            rows = np.concatenate(
                [dA[b * capA:(b + 1) * capA], dB[b * capB:(b + 1) * capB]],
                axis=0)                                # [nblkb, 128]
            dlocT[c, :, base:base + nblkb * 128] = np.broadcast_to(
                rows.reshape(-1).astype(np.int8), (128, nblkb * 128))

        xs = x[c * NPC:(c + 1) * NPC].astype(BF16)
        xT[c, :, :NPC] = xs.T

    iota = np.tile(np.arange(128, dtype=np.float32)[None, :], (128, 1)).astype(BF16)
    iotaP = np.arange(128, dtype=np.int8).reshape(128, 1)
    iotaPb = np.arange(128, dtype=np.float32).reshape(128, 1).astype(BF16)
    return (wtb, idxA, idxB, dloc_u, dlocT, xT, iota, iotaP, iotaPb,
            capA, capB)


def _build_program(capA, capB):
    nblkb = capA + capB
    nblk = NBUCK * nblkb
    nblkA = NBUCK * capA
    nblkB = NBUCK * capB
    f32 = mybir.dt.float32
    bf16 = mybir.dt.bfloat16
    i16 = mybir.dt.int16
    i8 = mybir.dt.int8

    nc = bass.Bass(trn_type="TRN2", num_devices=NC, num_swdge_queues=NQ)
    xT_in = nc.declare_dram_parameter("xT", [IN, XT_PAD], bf16, isOutput=False)
    wtb_in = nc.declare_dram_parameter("wtb", [IN, P1COLS], bf16, isOutput=False)
    idxA_in = nc.declare_dram_parameter("idxA", [128, nblkA * 8], i16, isOutput=False)
    idxB_in = nc.declare_dram_parameter("idxB", [128, nblkB * 8], i16, isOutput=False)
    dloc_in = nc.declare_dram_parameter("dloc", [128, nblk], bf16, isOutput=False)
    dlocT_in = nc.declare_dram_parameter("dlocT", [128, nblk * 128], i8, isOutput=False)
    iota_in = nc.declare_dram_parameter("iota", [128, 128], bf16, isOutput=False)
    iotaP_in = nc.declare_dram_parameter("iotaP", [128, 1], i8, isOutput=False)
    iotaPb_in = nc.declare_dram_parameter("iotaPb", [128, 1], bf16, isOutput=False)
    out_ext = nc.declare_dram_parameter("out", [NPC, HC], bf16, isOutput=True)

    tbl_locA = nc.dram_tensor("tbl_locA", [ROWS_A, TROW], bf16)
    tbl_locB = nc.dram_tensor("tbl_locB", [ROWS_B, TROW], bf16)
    tblA = nc.dram_tensor("tblA", [NC * ROWS_A, TROW], bf16, addr_space="Shared")
    tblB = nc.dram_tensor("tblB", [NC * ROWS_B, TROW], bf16, addr_space="Shared")
    er_tbl = nc.dram_tensor("er_tbl", [NBUCK * 128, H], bf16)

    with tile.TileContext(nc) as tc:
        with tc.tile_pool(name="cst", bufs=1) as cst:
            # ---------------- phase 1: Wh1 / el / er ----------------
            with tc.tile_pool(name="p1w", bufs=1) as p1w, \
                 tc.tile_pool(name="p1", bufs=3) as p1, \
                 tc.tile_pool(name="ps1", bufs=2, space="PSUM") as ps1:
                xts = []
                wtbs = []
                for k in range(2):
                    t = p1w.tile([128, XT_PAD], bf16, tag=f"xt{k}")
                    nc.sync.dma_start(out=t[:], in_=xT_in[k * 128:(k + 1) * 128, :])
                    xts.append(t)
                    u = p1w.tile([128, P1COLS], bf16, tag=f"wtb{k}")
                    nc.sync.dma_start(out=u[:], in_=wtb_in[k * 128:(k + 1) * 128, :])
                    wtbs.append(u)
                for tn in range(NBUCK):
                    ps = ps1.tile([128, P1COLS], f32)
                    for k in range(2):
                        nc.tensor.matmul(
                            out=ps[:],
                            lhsT=xts[k][:, tn * 128:(tn + 1) * 128],
                            rhs=wtbs[k][:],
                            start=(k == 0), stop=(k == 1),
                        )
                    sb = p1.tile([128, P1COLS], bf16)
                    nc.scalar.activation(out=sb[:], in_=ps[:],
                                         func=mybir.ActivationFunctionType.Copy)
                    sb4 = sb[:, 0:WH1].rearrange("p (h c) -> p h c", c=C1)
                    nc.vector.memset(sb4[:, :, C:C1], 1.0)
                    rows = min(128, NPC - tn * 128)
                    if tn < NBUCK_A:
                        trow = tbl_locA[tn * 128:tn * 128 + rows, 0:PAY]
                    else:
                        r0 = tn * 128 - ROWS_A
                        trow = tbl_locB[r0:r0 + rows, 0:PAY]
                    nc.sync.dma_start(out=trow, in_=sb[:rows, 0:PAY])
                    nc.sync.dma_start(
                        out=er_tbl[tn * 128:tn * 128 + rows, :],
                        in_=sb[:rows, PAY:P1COLS])

            # ---------------- all-gather both half tables ----------------
            nc.gpsimd.collective_compute(
                "AllGather", mybir.AluOpType.bypass,
                replica_groups=[list(range(NC))],
                ins=[tbl_locA[:].opt()], outs=[tblA[:].opt()],
            )
            nc.gpsimd.collective_compute(
                "AllGather", mybir.AluOpType.bypass,
                replica_groups=[list(range(NC))],
                ins=[tbl_locB[:].opt()], outs=[tblB[:].opt()],
            )
            nc.gpsimd.load_library(library_config.mlp)

            iota_b = cst.tile([128, 128], bf16)
            nc.sync.dma_start(out=iota_b[:], in_=iota_in[:, :])
            iota_m = iota_b[:].rearrange("p (b n) -> p b n", b=1)
            iotaP_sb = cst.tile([128, 1], i8)
            nc.sync.dma_start(out=iotaP_sb[:], in_=iotaP_in[:, :])
            iotaP_m = iotaP_sb[:].rearrange("p (b n) -> p b n", b=1)
            iotaPb_sb = cst.tile([128, 1], bf16)
            nc.sync.dma_start(out=iotaPb_sb[:], in_=iotaPb_in[:, :])
            ident = cst.tile([128, 128], bf16)
            nc.vector.tensor_tensor(
                out=ident[:], in0=iotaPb_sb[:].to_broadcast([128, 128]),
                in1=iota_b[:], op=mybir.AluOpType.is_equal)

            idxA_sb = cst.tile([128, nblkA * 8], i16)
            nc.sync.dma_start(out=idxA_sb[:], in_=idxA_in[:, :])
            idxB_sb = cst.tile([128, nblkB * 8], i16)
            nc.sync.dma_start(out=idxB_sb[:], in_=idxB_in[:, :])
            dloc_sb = cst.tile([128, nblk], bf16)
            nc.sync.dma_start(out=dloc_sb[:], in_=dloc_in[:, :])
            # er for all buckets, bucket-partition layout: [p, b, h]
            er_full = cst.tile([128, NBUCK * H], bf16)
            er_full3 = er_full[:].rearrange("p (b h) -> p b h", h=H)
            er_in3 = er_tbl[:(NBUCK - 1) * 128].rearrange("(b p) h -> p b h", p=128)
            nc.sync.dma_start(out=er_full3[:, 0:NBUCK - 1, :], in_=er_in3[:, :, :])
            # last bucket is partial (106 rows): zero first so the OTT matmul
            # can't pick up non-finite garbage from the dead partitions
            nc.vector.memset(er_full3[:, NBUCK - 1, :], 0.0)
            lastr = NPC - (NBUCK - 1) * 128
            nc.sync.dma_start(
                out=er_full3[0:lastr, NBUCK - 1, :],
                in_=er_tbl[(NBUCK - 1) * 128:NPC, :])

            # ---------------- er expansion on PE (overlaps AllGather) ------
            er_e = cst.tile([128, nblk * 8], bf16)   # SC-major slot order
            with tc.tile_pool(name="otq", bufs=2) as otq, \
                 tc.tile_pool(name="dtq", bufs=2) as dtq, \
                 tc.tile_pool(name="pse", bufs=2, space="PSUM") as pse, \
                 tc.tile_pool(name="gp", bufs=4) as gp, \
                 tc.tile_pool(name="vp", bufs=2) as vp, \
                 tc.tile_pool(name="otp", bufs=2) as otp, \
                 tc.tile_pool(name="sp", bufs=2) as sp, \
                 tc.tile_pool(name="sf", bufs=3) as sf, \
                 tc.tile_pool(name="np_", bufs=3) as np_, \
                 tc.tile_pool(name="ps2", bufs=4, space="PSUM") as ps2p:
                n_sc = (NBUCK + SC_BUCKETS - 1) // SC_BUCKETS
                for b in range(NBUCK):
                    dT = dtq.tile([128, nblkb * 128], i8, tag="dT")
                    nc.sync.dma_start(
                        out=dT[:],
                        in_=dlocT_in[:, b * nblkb * 128:(b + 1) * nblkb * 128])
                    dT3 = dT[:].rearrange("p (j n) -> p j n", n=128)
                    OTT = otq.tile([128, nblkb * 128], bf16, tag="OTT")
                    OTT3 = OTT[:].rearrange("p (j n) -> p j n", n=128)
                    nc.vector.tensor_tensor(
                        out=OTT3, in0=iotaP_m.to_broadcast([128, nblkb, 128]),
                        in1=dT3, op=mybir.AluOpType.is_equal)
                    pe = pse.tile([128, nblkb * H], f32)
                    pe3 = pe[:].rearrange("p (j h) -> p j h", h=H)
                    for j in range(nblkb):
                        nc.tensor.matmul(
                            out=pe3[:, j, :], lhsT=OTT3[:, j, :],
                            rhs=er_full3[:, b, :], start=True, stop=True)
                    # copy into er_e at SC-major positions (A-run | B-run)
                    sc = b // SC_BUCKETS
                    b0 = sc * SC_BUCKETS
                    nbk = min(SC_BUCKETS, NBUCK - b0)
                    off_sc = b0 * nblkb
                    offA = (off_sc + (b - b0) * capA) * 8
                    offB = (off_sc + nbk * capA + (b - b0) * capB) * 8
                    nc.scalar.activation(
                        out=er_e[:, offA:offA + capA * 8],
                        in_=pe[:, 0:capA * 8],
                        func=mybir.ActivationFunctionType.Copy)
                    nc.scalar.activation(
                        out=er_e[:, offB:offB + capB * 8],
                        in_=pe[:, capA * 8:nblkb * 8],
                        func=mybir.ActivationFunctionType.Copy)

                # ---------------- phase 2: gather / score / scatter --------
                pend = {}
                qctr = [0]

                def next_q():
                    q = qctr[0] % NQ
                    qctr[0] += 1
                    return q

                def emit_A(sc):
                    b0 = sc * SC_BUCKETS
                    nbk = min(SC_BUCKETS, NBUCK - b0)
                    nbA = nbk * capA
                    nbB = nbk * capB
                    nb = nbA + nbB
                    G = gp.tile([128, nb * GELEM], bf16, tag="G")
                    G3 = G[:].rearrange("p (b y) -> p b y", y=GELEM)
                    a0 = b0 * capA * 8
                    _dma_gather_raw(
                        nc.gpsimd, G3[:, 0:nbA, :], tblA[:],
                        idxA_sb[:, a0:a0 + nbA * 8], nbA * 128,
                        GELEM, TROW, queue_num=next_q())
                    pend[sc] = (G3, b0, nbk, nbA, nbB, nb)

                def emit_B(sc):
                    G3, b0, nbk, nbA, nbB, nb = pend[sc]
                    bb0 = b0 * capB * 8
                    _dma_gather_raw(
                        nc.gpsimd, G3[:, nbA:nb, :], tblB[:],
                        idxB_sb[:, bb0:bb0 + nbB * 8], nbB * 128,
                        GELEM, TROW, queue_num=next_q())

                def emit_compute(sc):
                    G3, b0, nbk, nbA, nbB, nb = pend.pop(sc)
                    off_u = b0 * nblkb
                    # scores: e = el + er ; leaky ; exp
                    e_t = sp.tile([128, nb * H], f32, tag="e")
                    e3 = e_t[:].rearrange("p (b h) -> p b h", h=H)
                    er_sc3 = er_e[:, off_u * 8:(off_u + nb) * 8].rearrange(
                        "p (b h) -> p b h", h=H)
                    nc.vector.tensor_tensor(
                        out=e3, in0=G3[:, :, WH1:PAY], in1=er_sc3,
                        op=mybir.AluOpType.add)
                    # w = exp(leaky(e)) = max(exp(e), exp(NEG*e)) (exp monotone)
                    w_t = sp.tile([128, nb * H], bf16, tag="w")
                    w1_t = sp.tile([128, nb * H], bf16, tag="w1")
                    nc.scalar.activation(
                        out=w1_t[:], in_=e_t[:],
                        func=mybir.ActivationFunctionType.Exp)
                    w2_t = sp.tile([128, nb * H], bf16, tag="w2")
                    nc.scalar.activation(
                        out=w2_t[:], in_=e_t[:], scale=NEG,
                        func=mybir.ActivationFunctionType.Exp)
                    nc.vector.tensor_tensor(
                        out=w_t[:], in0=w1_t[:], in1=w2_t[:],
                        op=mybir.AluOpType.max)
                    w3 = w_t[:].rearrange("p (b h) -> p b h", h=H)

                    # one-op one-hot build for the whole super-chunk
                    OT = otp.tile([128, nb * 128], bf16, tag="OT")
                    OT3 = OT[:].rearrange("p (b n) -> p b n", n=128)
                    d3 = dloc_sb[:, off_u:off_u + nb].to_broadcast([128, nb, 128])
                    i3 = iota_m.to_broadcast([128, nb, 128])
                    nc.vector.tensor_tensor(out=OT3, in0=d3, in1=i3,
                                            op=mybir.AluOpType.is_equal)

                    # fused V multiply for the whole super-chunk
                    V = vp.tile([128, nb * WH1], bf16, tag="V")
                    V3 = V[:].rearrange("p (b y) -> p b y", y=WH1)
                    V4 = V[:].rearrange("p (b h c) -> p b h c", h=H, c=C1)
                    G4 = G3[:, :, 0:WH1].rearrange("p b (h c) -> p b h c", c=C1)
                    w4 = w3.to_broadcast([128, nb, H, C1])
                    nc.vector.tensor_tensor(out=V4, in0=G4, in1=w4,
                                            op=mybir.AluOpType.mult)

                    # per bucket: scatter matmuls + self-loop fold + normalize
                    for bb in range(nbk):
                        bucket = b0 + bb
                        # self-loop contribution
                        ts = sf.tile([128, PAY], bf16, tag="ts")
                        if bucket < NBUCK_A:
                            tsrc = tbl_locA[bucket * 128:bucket * 128 + 128, 0:PAY]
                        else:
                            r0 = bucket * 128 - ROWS_A
                            rows = min(128, ROWS_B - r0)
                            tsrc = tbl_locB[r0:r0 + rows, 0:PAY]
                        nc.sync.dma_start(out=ts[:tsrc.shape[0], :], in_=tsrc)
                        es_s = sf.tile([128, H], f32, tag="es_s")
                        nc.vector.tensor_tensor(
                            out=es_s[:], in0=ts[:, WH1:PAY],
                            in1=er_full3[:, bucket, :], op=mybir.AluOpType.add)
                        ws_s = sf.tile([128, H], bf16, tag="ws_s")
                        ws1 = sf.tile([128, H], bf16, tag="ws1")
                        nc.scalar.activation(
                            out=ws1[:], in_=es_s[:],
                            func=mybir.ActivationFunctionType.Exp)
                        ws2 = sf.tile([128, H], bf16, tag="ws2")
                        nc.scalar.activation(
                            out=ws2[:], in_=es_s[:], scale=NEG,
                            func=mybir.ActivationFunctionType.Exp)
                        nc.vector.tensor_tensor(
                            out=ws_s[:], in0=ws1[:], in1=ws2[:],
                            op=mybir.AluOpType.max)
                        vs = sf.tile([128, WH1], bf16, tag="vs")
                        vs4 = vs[:].rearrange("p (h c) -> p h c", c=C1)
                        ts4 = ts[:, 0:WH1].rearrange("p (h c) -> p h c", c=C1)
                        nc.vector.tensor_tensor(
                            out=vs4, in0=ts4,
                            in1=ws_s[:].to_broadcast([128, H, C1]),
                            op=mybir.AluOpType.mult)

                        blks = ([bb * capA + j for j in range(capA)] +
                                [nbA + bb * capB + j for j in range(capB)])
                        ps = ps2p.tile([128, WH1], f32)
                        for i, blk in enumerate(blks):
                            nc.tensor.matmul(
                                out=ps[:], lhsT=OT3[:, blk, :], rhs=V3[:, blk, :],
                                start=(i == 0), stop=False)
                        nc.tensor.matmul(
                            out=ps[:], lhsT=ident[:], rhs=vs[:],
                            start=False, stop=True)

                        ps4 = ps[:].rearrange("p (h c) -> p h c", c=C1)
                        den = np_.tile([128, H], f32, tag="den")
                        nc.vector.tensor_scalar_add(den[:], ps4[:, :, C], EPS)
                        rec = np_.tile([128, H], f32, tag="rec")
                        nc.vector.reciprocal(rec[:], den[:])
                        ot = np_.tile([128, HC], bf16, tag="ot")
                        ot3 = ot[:].rearrange("p (h c) -> p h c", c=C)
                        r3 = rec[:].to_broadcast([128, H, C])
                        nc.vector.tensor_tensor(out=ot3, in0=ps4[:, :, 0:C],
                                                in1=r3, op=mybir.AluOpType.mult)
                        rows = min(128, NPC - bucket * 128)
                        nc.sync.dma_start(
                            out=out_ext[bucket * 128:bucket * 128 + rows, :],
                            in_=ot[:rows, :])

                LAG = 3
                for sc in range(n_sc):
                    emit_A(sc)
                    if sc >= LAG:
                        emit_B(sc - LAG)
                        emit_compute(sc - LAG)
                for sc in range(max(0, n_sc - LAG), n_sc):
                    emit_B(sc)
                    emit_compute(sc)

    _split_excess_waits(nc)
    _move_reload_after_collectives(nc)
    lower_extended_insts(nc)
    return nc


def kernel(**inputs):
    x = np.asarray(inputs["x"], np.float32)
    edge_index = np.asarray(inputs["edge_index"])
    W = np.asarray(inputs["W"], np.float32)
    a_left = np.asarray(inputs["a_left"], np.float32)
    a_right = np.asarray(inputs["a_right"], np.float32)

    (wtb, idxA, idxB, dloc_u, dlocT, xT, iota, iotaP, iotaPb,
     capA, capB) = _host_prep(x, edge_index, W, a_left, a_right)
    nc = _build_program(capA, capB)

    in_maps = []
    for c in range(NC):
        in_maps.append({
            "xT": np.ascontiguousarray(xT[c]),
            "wtb": wtb,
            "idxA": np.ascontiguousarray(idxA[c]),
            "idxB": np.ascontiguousarray(idxB[c]),
            "dloc": np.ascontiguousarray(dloc_u[c]),
            "dlocT": np.ascontiguousarray(dlocT[c]),
            "iota": iota,
            "iotaP": iotaP,
            "iotaPb": iotaPb,
        })

    res = run_bass_kernel_spmd(nc, in_maps, core_ids=list(range(NC)))
    out = np.concatenate([np.asarray(res.results[c]["out"]).astype(np.float32)
                          for c in range(NC)], axis=0)
    return out



# revision 9
# speedup vs baseline: 1.4903x; 1.0658x over previous
"""GAT layer on 8 Trainium2 NeuronCores (Bass/Tile), edge-parallel dst-sharded.

Self-contained. Host preprocesses the graph (dst-shard, bucket sort with
uniform caps, A/B split of each bucket's edges by source half for int16
gather indices; self-loops are NOT materialized as edges). Device program:

  phase 1: per 128-node tile, matmul x @ [W.T | a-folded] producing rows
    [Wh1(264) | el(8) | er(8)]; Wh1 interleaves a constant 1.0 after each
    head's 32 channels so a single multiply by w=exp(leaky(e)) yields both
    the weighted message AND the softmax-denominator column. Rows go to two
    local half-tables (stride 384 for dma_gather's 256B-stride rule) and a
    compact local er table.
  AllGather x2: half tables -> tblA/tblB on every core (AG1 fires once the
    first 25 buckets are written and overlaps the rest of phase 1).
  er expansion on the TENSOR engine (no DMA gather): per bucket, a
    transposed one-hot OTT (built from host-shipped per-slot dst indices)
    times the bucket's er rows gives per-edge-slot er for every block -
    packed into one PSUM tile per bucket and copied out. Runs during the
    AllGather window.
  phase 2 per super-chunk: two dma_gather calls (A/B) fetch table rows by
    src; score ops (add on DVE, leaky+exp on the scalar engine); one-op
    one-hot build; per-block V multiplies; per bucket capA+capB one-hot
    scatter matmuls accumulated in PSUM, then a final identity-matmul adds
    the analytically-computed self-loop contribution, and the bucket is
    normalized by the gathered denominator column and written out.

The SWDGE (GpSimd) descriptor path only carries the unavoidable per-edge
table gather; everything index-like that is bucket-local (er by dst, the
self loops) rides the tensor engine instead.
"""
import sys

for _p in ("/opt/trn_rl_repo",):
    if _p not in sys.path:
        sys.path.insert(0, _p)

import numpy as np
import ml_dtypes

import concourse.bass as bass
import concourse.tile as tile
from concourse import mybir, library_config
from concourse.bass_utils import run_bass_kernel_spmd
from concourse.ap_utils import ap_is_contiguous
from concourse.library_overlay import lower_extended_insts

BF16 = ml_dtypes.bfloat16

N = 50000
E = 800000
IN = 256
H = 8
C = 32
C1 = C + 1            # 33: [Wh_h(32) | 1]
HC = H * C            # 256
WH1 = H * C1          # 264
NC = 8
NPC = N // NC         # 6250 nodes per core
BUCKET = 128
NBUCK = (NPC + BUCKET - 1) // BUCKET   # 49
XT_PAD = NBUCK * 128                   # 6272
PAY = WH1 + H         # 272: gather payload [Wh1(264) | el(8)]
P1COLS = PAY + H      # 280: phase-1 matmul out [Wh1 | el | er]
TROW = 384            # table row stride (256B multiple)
GELEM = 272           # gathered elements per row
NEG = 0.2
EPS = 1e-16
SC_BUCKETS = 2        # buckets per gather super-chunk
NBUCK_A = 25          # buckets 0..24 -> table half A
ROWS_A = NBUCK_A * BUCKET          # 3200
ROWS_B = NPC - ROWS_A              # 3050
USE_PREP_TRIGGER = False
USE_ACT_LRELU = False
NQ = 4

_waitfix_ctr = [0]
_reg_cache = {}


def _split_excess_waits(nc, max_waits=1):
    # walrus in this container caps sync waits per instruction at 1; hoist
    # excess onto same-engine NoOps.
    n_fixed = 0
    for fn in nc.m.functions:
        for bb in fn.blocks:
            insts = bb.instructions
            out = []
            for ins in insts:
                si = ins.sync_info
                waits = list(si.on_wait) if si is not None and si.on_wait else []
                if len(waits) > max_waits:
                    keep = waits[-max_waits:]
                    extra = waits[:-max_waits]
                    for i in range(0, len(extra), max_waits):
                        grp = extra[i:i + max_waits]
                        _waitfix_ctr[0] += 1
                        nop = mybir.InstNoOp(
                            name=f"I-waitfix-{_waitfix_ctr[0]}", ins=[], outs=[])
                        nop.engine = ins.engine
                        nop.sync_info = mybir.SyncInfo(on_wait=grp, on_update=[])
                        nc.register_instruction(nop)
                        out.append(nop)
                    si.on_wait = keep
                    n_fixed += 1
                out.append(ins)
            if len(out) != len(insts):
                bb.instructions = out
    return n_fixed


def _move_reload_after_collectives(nc):
    """The tile scheduler floats the dependency-less library-reload pseudo to
    the top of the program; keep it after the last collective trigger."""
    from concourse import bass_isa
    for fn in nc.m.functions:
        for bb in fn.blocks:
            insts = bb.instructions
            reload_idx = [i for i, ins in enumerate(insts)
                          if isinstance(ins, bass_isa.InstPseudoReloadLibraryIndex)]
            coll_idx = [i for i, ins in enumerate(insts)
                        if isinstance(ins, mybir.InstCollectiveCompute)]
            if not reload_idx or not coll_idx:
                continue
            assert len(reload_idx) == 1
            r = reload_idx[0]
            last_c = max(coll_idx)
            if r > last_c:
                continue
            ins = insts.pop(r)
            insts.insert(last_c, ins)
            bb.instructions = insts
    return nc


def _dma_gather_raw(eng, out_ap, in_ap, idxs_ap, num_idxs, elem_size, elem_step,
                    sem=None, queue_num=0):
    """bass.dma_gather without the elem_size_bytes%256 assert (non-transpose,
    DRAM source, 256B-multiple row stride). sem!=None -> prepare_only."""
    assert idxs_ap.dtype == mybir.dt.int16
    assert in_ap.dtype == out_ap.dtype
    assert ap_is_contiguous(out_ap.ap[1:])
    assert ap_is_contiguous(idxs_ap.ap[1:])
    assert in_ap.ap[0][0] == elem_step
    stride_bytes = elem_step * mybir.dt.size(in_ap.dtype)
    stride_bytes_256 = stride_bytes // 256
    assert stride_bytes_256 * 256 == stride_bytes and stride_bytes_256 < 256
    _in_ap = eng.lower_ap_dma(in_ap, for_custom_bir_dma=True)
    _idxs_ap = eng.lower_ap(idxs_ap)
    _out_ap = eng.lower_ap(out_ap)
    key = (id(eng.bass), num_idxs)
    if key not in _reg_cache:
        _reg_cache[key] = eng.to_reg(num_idxs)
    inst = eng.add_instruction(
        mybir.InstDMAGatherAnt(
            name=eng.bass.get_next_instruction_name(),
            ins=[*_in_ap, _idxs_ap, eng.lower_val_access(_reg_cache[key])],
            outs=[_out_ap],
            transpose=False,
            num_idxs=num_idxs,
            elem_size=elem_size,
            stride_bytes_256=stride_bytes_256,
            gen_mode=int(sem is not None),
            single_packet=False,
            queue_num=queue_num,
            sbuf_tokens_per_rank=0,
            sbuf_free_dim_per_rank=0,
            sbuf_free_dim_pad_per_rank=0,
            sbuf_byte_offset=0,
        )
    )
    if sem is not None:
        inst.then_inc(sem, 16)
        return eng._track_prepare_only(inst, queue_num)
    return inst


def _wrap16(vals):
    """Edge-slot int16 index array -> dma_gather layout [128, n/16]."""
    n = len(vals)
    assert n % 16 == 0
    w = np.asarray(vals, np.int16).reshape(n // 16, 16).T
    return np.tile(w, (8, 1))


def _host_prep(x, edge_index, W, a_left, a_right):
    src = np.asarray(edge_index[0], np.int64)
    dst = np.asarray(edge_index[1], np.int64)

    # fold attention vectors through W:  [el|er] = x @ (W.T @ A)
    A = np.zeros((HC, 2 * H), np.float32)
    for h in range(H):
        A[h * C:(h + 1) * C, h] = a_left[h]
        A[h * C:(h + 1) * C, H + h] = a_right[h]
    B = (W.T.astype(np.float64) @ A.astype(np.float64)).astype(np.float32)
    wtb = np.zeros((IN, P1COLS), np.float32)
    for h in range(H):
        wtb[:, h * C1:h * C1 + C] = W.T[:, h * C:(h + 1) * C]
    wtb[:, WH1:WH1 + H] = B[:, :H]
    wtb[:, WH1 + H:] = B[:, H:]
    wtb = wtb.astype(BF16)

    core = dst // NPC
    r_src = src % NPC
    c_src = src // NPC
    is_a = r_src < ROWS_A
    gidx = np.where(is_a, c_src * ROWS_A + r_src,
                    c_src * ROWS_B + (r_src - ROWS_A)).astype(np.int64)

    capA = capB = 0
    lists = {}
    for c in range(NC):
        m = core == c
        s_c, d_c, g_c, a_c = src[m], dst[m], gidx[m], is_a[m]
        dl = d_c - c * NPC
        b_c = dl // BUCKET
        order = np.lexsort((s_c, b_c))
        s_c, dl, b_c, g_c, a_c = (s_c[order], dl[order], b_c[order],
                                  g_c[order], a_c[order])
        cnt = np.bincount(b_c, minlength=NBUCK)
        starts = np.concatenate([[0], np.cumsum(cnt)[:-1]])
        for b in range(NBUCK):
            sl = slice(starts[b], starts[b] + cnt[b])
            aa = a_c[sl]
            lists[(c, b)] = (g_c[sl][aa], dl[sl][aa] - b * BUCKET,
                             g_c[sl][~aa], dl[sl][~aa] - b * BUCKET)
            capA = max(capA, (int(aa.sum()) + 127) // 128)
            capB = max(capB, (int((~aa).sum()) + 127) // 128)

    nblkb = capA + capB
    nblk = NBUCK * nblkb
    nblkA = NBUCK * capA
    nblkB = NBUCK * capB
    n_sc = (NBUCK + SC_BUCKETS - 1) // SC_BUCKETS

    idxA = np.zeros((NC, 128, nblkA * 8), np.int16)
    idxB = np.zeros((NC, 128, nblkB * 8), np.int16)
    dloc_u = np.full((NC, 128, nblk), 200.0, BF16)
    dlocT = np.full((NC, 128, nblk * 128), -1, np.int8)
    xT = np.zeros((NC, IN, XT_PAD), BF16)

    for c in range(NC):
        iA = np.zeros(nblkA * 128, np.int64)
        iB = np.zeros(nblkB * 128, np.int64)
        dA = np.full((nblkA, 128), -1, np.int64)
        dB = np.full((nblkB, 128), -1, np.int64)
        for b in range(NBUCK):
            gA, dlA, gB, dlB = lists[(c, b)]
            oa = b * capA * 128
            ob = b * capB * 128
            iA[oa:oa + len(gA)] = gA
            iB[ob:ob + len(gB)] = gB
            fa = dA[b * capA:(b + 1) * capA].reshape(-1)
            fa[:len(dlA)] = dlA
            fb = dB[b * capB:(b + 1) * capB].reshape(-1)
            fb[:len(dlB)] = dlB
        idxA[c] = _wrap16(iA)
        idxB[c] = _wrap16(iB)
        # slot k of block j = (partition k%128); dA rows are flat slot runs
        dA = dA.reshape(nblkA, 128)
        dB = dB.reshape(nblkB, 128)
        # dloc_u: SC-major order [per SC: A-blocks | B-blocks], [128, nblk]
        off = 0
        for sc in range(n_sc):
            b0 = sc * SC_BUCKETS
            bs = range(b0, min(b0 + SC_BUCKETS, NBUCK))
            for b in bs:
                blk = dA[b * capA:(b + 1) * capA]     # [capA, 128]
                v = np.where(blk < 0, 200.0, blk).astype(np.float32)
                dloc_u[c, :, off:off + capA] = v.T.astype(BF16)
                off += capA
            for b in bs:
                blk = dB[b * capB:(b + 1) * capB]
                v = np.where(blk < 0, 200.0, blk).astype(np.float32)
                dloc_u[c, :, off:off + capB] = v.T.astype(BF16)
                off += capB
        # dlocT: BUCKET-major order [per bucket: A-blocks | B-blocks],
        # transposed and replicated: [128(any), (b*nblkb + j)*128 + p]
        for b in range(NBUCK):
            base = b * nblkb * 128
            rows = np.concatenate(
                [dA[b * capA:(b + 1) * capA], dB[b * capB:(b + 1) * capB]],
                axis=0)                                # [nblkb, 128]
            dlocT[c, :, base:base + nblkb * 128] = np.broadcast_to(
                rows.reshape(-1).astype(np.int8), (128, nblkb * 128))

        xs = x[c * NPC:(c + 1) * NPC].astype(BF16)
        xT[c, :, :NPC] = xs.T

    iota = np.tile(np.arange(128, dtype=np.float32)[None, :], (128, 1)).astype(BF16)
    iotaP = np.arange(128, dtype=np.int8).reshape(128, 1)
    iotaPb = np.arange(128, dtype=np.float32).reshape(128, 1).astype(BF16)
    return (wtb, idxA, idxB, dloc_u, dlocT, xT, iota, iotaP, iotaPb,
            capA, capB)


def _build_program(capA, capB):
    nblkb = capA + capB
    nblk = NBUCK * nblkb
    nblkA = NBUCK * capA
    nblkB = NBUCK * capB
    f32 = mybir.dt.float32
    bf16 = mybir.dt.bfloat16
    i16 = mybir.dt.int16
    i8 = mybir.dt.int8

    nc = bass.Bass(trn_type="TRN2", num_devices=NC, num_swdge_queues=NQ)
    xT_in = nc.declare_dram_parameter("xT", [IN, XT_PAD], bf16, isOutput=False)
    wtb_in = nc.declare_dram_parameter("wtb", [IN, P1COLS], bf16, isOutput=False)
    idxA_in = nc.declare_dram_parameter("idxA", [128, nblkA * 8], i16, isOutput=False)
    idxB_in = nc.declare_dram_parameter("idxB", [128, nblkB * 8], i16, isOutput=False)
    dloc_in = nc.declare_dram_parameter("dloc", [128, nblk], bf16, isOutput=False)
    dlocT_in = nc.declare_dram_parameter("dlocT", [128, nblk * 128], i8, isOutput=False)
    iota_in = nc.declare_dram_parameter("iota", [128, 128], bf16, isOutput=False)
    iotaP_in = nc.declare_dram_parameter("iotaP", [128, 1], i8, isOutput=False)
    iotaPb_in = nc.declare_dram_parameter("iotaPb", [128, 1], bf16, isOutput=False)
    out_ext = nc.declare_dram_parameter("out", [NPC, HC], bf16, isOutput=True)

    tbl_locA = nc.dram_tensor("tbl_locA", [ROWS_A, TROW], bf16)
    tbl_locB = nc.dram_tensor("tbl_locB", [ROWS_B, TROW], bf16)
    tblA = nc.dram_tensor("tblA", [NC * ROWS_A, TROW], bf16, addr_space="Shared")
    tblB = nc.dram_tensor("tblB", [NC * ROWS_B, TROW], bf16, addr_space="Shared")

    with tile.TileContext(nc) as tc:
        with tc.tile_pool(name="cst", bufs=1) as cst:
            # er for all buckets, bucket-partition layout [p, b, h]; filled
            # directly from phase-1 PSUM (dead tail rows produce er=0)
            er_full = cst.tile([128, NBUCK * H], bf16)
            er_full3 = er_full[:].rearrange("p (b h) -> p b h", h=H)
            # ---------------- phase 1: Wh1 / el / er ----------------
            with tc.tile_pool(name="p1w", bufs=1) as p1w, \
                 tc.tile_pool(name="p1", bufs=3) as p1, \
                 tc.tile_pool(name="ps1", bufs=2, space="PSUM") as ps1:
                xts = []
                wtbs = []
                for k in range(2):
                    t = p1w.tile([128, XT_PAD], bf16, tag=f"xt{k}")
                    nc.sync.dma_start(out=t[:], in_=xT_in[k * 128:(k + 1) * 128, :])
                    xts.append(t)
                    u = p1w.tile([128, P1COLS], bf16, tag=f"wtb{k}")
                    nc.sync.dma_start(out=u[:], in_=wtb_in[k * 128:(k + 1) * 128, :])
                    wtbs.append(u)
                groups = []
                for b0s, b1s in ((0, NBUCK_A), (NBUCK_A, NBUCK - 1)):
                    b = b0s
                    while b < b1s:
                        g = min(4, b1s - b)
                        groups.append((b, g))
                        b += g
                groups.append((NBUCK - 1, 1))
                BANK = 512          # f32 elems per PSUM bank
                for b0, g in groups:
                    ps = ps1.tile([128, 4 * BANK], f32)
                    ps3 = ps[:].rearrange("p (g y) -> p g y", y=BANK)
                    for i in range(g):
                        tn = b0 + i
                        for k in range(2):
                            nc.tensor.matmul(
                                out=ps3[:, i, 0:P1COLS],
                                lhsT=xts[k][:, tn * 128:(tn + 1) * 128],
                                rhs=wtbs[k][:],
                                start=(k == 0), stop=(k == 1),
                            )
                    sb = p1.tile([128, 4 * P1COLS], bf16)
                    sb3g = sb[:, 0:g * P1COLS].rearrange("p (g y) -> p g y", y=P1COLS)
                    nc.scalar.activation(out=sb3g,
                                         in_=ps3[:, 0:g, 0:P1COLS],
                                         func=mybir.ActivationFunctionType.Copy)
                    sb4 = sb[:, 0:g * P1COLS].rearrange(
                        "p (g y) -> p g y", y=P1COLS)[:, :, 0:WH1].rearrange(
                        "p g (h c) -> p g h c", c=C1)
                    nc.vector.memset(sb4[:, :, :, C:C1], 1.0)
                    nc.scalar.activation(out=er_full3[:, b0:b0 + g, :],
                                         in_=ps3[:, 0:g, PAY:P1COLS],
                                         func=mybir.ActivationFunctionType.Copy)
                    sb3 = sb[:, 0:g * P1COLS].rearrange("p (g y) -> p g y", y=P1COLS)
                    if b0 + g <= NBUCK_A:
                        dst = tbl_locA[b0 * 128:(b0 + g) * 128, 0:PAY]
                    else:
                        r0 = b0 * 128 - ROWS_A
                        rows = min(g * 128, ROWS_B - r0)
                        dst = tbl_locB[r0:r0 + rows, 0:PAY]
                    if dst.shape[0] == g * 128:
                        dst3 = dst.rearrange("(g p) y -> p g y", p=128)
                        nc.sync.dma_start(out=dst3, in_=sb3[:, :, 0:PAY])
                    else:
                        nc.sync.dma_start(out=dst, in_=sb[:dst.shape[0], 0:PAY])

            # ---------------- all-gather both half tables ----------------
            nc.gpsimd.collective_compute(
                "AllGather", mybir.AluOpType.bypass,
                replica_groups=[list(range(NC))],
                ins=[tbl_locA[:].opt()], outs=[tblA[:].opt()],
            )
            nc.gpsimd.collective_compute(
                "AllGather", mybir.AluOpType.bypass,
                replica_groups=[list(range(NC))],
                ins=[tbl_locB[:].opt()], outs=[tblB[:].opt()],
            )
            nc.gpsimd.load_library(library_config.mlp)

            iota_b = cst.tile([128, 128], bf16)
            nc.sync.dma_start(out=iota_b[:], in_=iota_in[:, :])
            iota_m = iota_b[:].rearrange("p (b n) -> p b n", b=1)
            iotaP_sb = cst.tile([128, 1], i8)
            nc.sync.dma_start(out=iotaP_sb[:], in_=iotaP_in[:, :])
            iotaP_m = iotaP_sb[:].rearrange("p (b n) -> p b n", b=1)
            iotaPb_sb = cst.tile([128, 1], bf16)
            nc.sync.dma_start(out=iotaPb_sb[:], in_=iotaPb_in[:, :])
            ident = cst.tile([128, 128], bf16)
            nc.vector.tensor_tensor(
                out=ident[:], in0=iotaPb_sb[:].to_broadcast([128, 128]),
                in1=iota_b[:], op=mybir.AluOpType.is_equal)

            idxA_sb = cst.tile([128, nblkA * 8], i16)
            nc.sync.dma_start(out=idxA_sb[:], in_=idxA_in[:, :])
            idxB_sb = cst.tile([128, nblkB * 8], i16)
            nc.sync.dma_start(out=idxB_sb[:], in_=idxB_in[:, :])
            dloc_sb = cst.tile([128, nblk], bf16)
            nc.sync.dma_start(out=dloc_sb[:], in_=dloc_in[:, :])
            # ---------------- er expansion on PE (overlaps AllGather) ------
            er_e = cst.tile([128, nblk * 8], bf16)   # SC-major slot order
            with tc.tile_pool(name="otq", bufs=2) as otq, \
                 tc.tile_pool(name="dtq", bufs=2) as dtq, \
                 tc.tile_pool(name="pse", bufs=2, space="PSUM") as pse, \
                 tc.tile_pool(name="gp", bufs=4) as gp, \
                 tc.tile_pool(name="vp", bufs=2) as vp, \
                 tc.tile_pool(name="otp", bufs=2) as otp, \
                 tc.tile_pool(name="sp", bufs=2) as sp, \
                 tc.tile_pool(name="sf", bufs=3) as sf, \
                 tc.tile_pool(name="np_", bufs=3) as np_, \
                 tc.tile_pool(name="ps2", bufs=4, space="PSUM") as ps2p:
                n_sc = (NBUCK + SC_BUCKETS - 1) // SC_BUCKETS
                for b in range(NBUCK):
                    dT = dtq.tile([128, nblkb * 128], i8, tag="dT")
                    nc.sync.dma_start(
                        out=dT[:],
                        in_=dlocT_in[:, b * nblkb * 128:(b + 1) * nblkb * 128])
                    dT3 = dT[:].rearrange("p (j n) -> p j n", n=128)
                    OTT = otq.tile([128, nblkb * 128], bf16, tag="OTT")
                    OTT3 = OTT[:].rearrange("p (j n) -> p j n", n=128)
                    nc.vector.tensor_tensor(
                        out=OTT3, in0=iotaP_m.to_broadcast([128, nblkb, 128]),
                        in1=dT3, op=mybir.AluOpType.is_equal)
                    pe = pse.tile([128, nblkb * H], f32)
                    pe3 = pe[:].rearrange("p (j h) -> p j h", h=H)
                    for j in range(nblkb):
                        nc.tensor.matmul(
                            out=pe3[:, j, :], lhsT=OTT3[:, j, :],
                            rhs=er_full3[:, b, :], start=True, stop=True)
                    # copy into er_e at SC-major positions (A-run | B-run)
                    sc = b // SC_BUCKETS
                    b0 = sc * SC_BUCKETS
                    nbk = min(SC_BUCKETS, NBUCK - b0)
                    off_sc = b0 * nblkb
                    offA = (off_sc + (b - b0) * capA) * 8
                    offB = (off_sc + nbk * capA + (b - b0) * capB) * 8
                    nc.scalar.activation(
                        out=er_e[:, offA:offA + capA * 8],
                        in_=pe[:, 0:capA * 8],
                        func=mybir.ActivationFunctionType.Copy)
                    nc.scalar.activation(
                        out=er_e[:, offB:offB + capB * 8],
                        in_=pe[:, capA * 8:nblkb * 8],
                        func=mybir.ActivationFunctionType.Copy)

                # ---------------- phase 2: gather / score / scatter --------
                pend = {}
                qctr = [0]

                def next_q():
                    q = qctr[0] % NQ
                    qctr[0] += 1
                    return q

                def emit_A(sc):
                    b0 = sc * SC_BUCKETS
                    nbk = min(SC_BUCKETS, NBUCK - b0)
                    nbA = nbk * capA
                    nbB = nbk * capB
                    nb = nbA + nbB
                    G = gp.tile([128, nb * GELEM], bf16, tag="G")
                    G3 = G[:].rearrange("p (b y) -> p b y", y=GELEM)
                    a0 = b0 * capA * 8
                    _dma_gather_raw(
                        nc.gpsimd, G3[:, 0:nbA, :], tblA[:],
                        idxA_sb[:, a0:a0 + nbA * 8], nbA * 128,
                        GELEM, TROW, queue_num=next_q())
                    pend[sc] = (G3, b0, nbk, nbA, nbB, nb)

                def emit_B(sc):
                    G3, b0, nbk, nbA, nbB, nb = pend[sc]
                    bb0 = b0 * capB * 8
                    _dma_gather_raw(
                        nc.gpsimd, G3[:, nbA:nb, :], tblB[:],
                        idxB_sb[:, bb0:bb0 + nbB * 8], nbB * 128,
                        GELEM, TROW, queue_num=next_q())

                def emit_compute(sc):
                    G3, b0, nbk, nbA, nbB, nb = pend.pop(sc)
                    off_u = b0 * nblkb
                    # scores: e = el + er ; leaky ; exp
                    e_t = sp.tile([128, nb * H], f32, tag="e")
                    e3 = e_t[:].rearrange("p (b h) -> p b h", h=H)
                    er_sc3 = er_e[:, off_u * 8:(off_u + nb) * 8].rearrange(
                        "p (b h) -> p b h", h=H)
                    nc.vector.tensor_tensor(
                        out=e3, in0=G3[:, :, WH1:PAY], in1=er_sc3,
                        op=mybir.AluOpType.add)
                    # w = exp(leaky(e)) = max(exp(e), exp(NEG*e)) (exp monotone)
                    w_t = sp.tile([128, nb * H], bf16, tag="w")
                    w1_t = sp.tile([128, nb * H], bf16, tag="w1")
                    nc.scalar.activation(
                        out=w1_t[:], in_=e_t[:],
                        func=mybir.ActivationFunctionType.Exp)
                    w2_t = sp.tile([128, nb * H], bf16, tag="w2")
                    nc.scalar.activation(
                        out=w2_t[:], in_=e_t[:], scale=NEG,
                        func=mybir.ActivationFunctionType.Exp)
                    nc.vector.tensor_tensor(
                        out=w_t[:], in0=w1_t[:], in1=w2_t[:],
                        op=mybir.AluOpType.max)
                    w3 = w_t[:].rearrange("p (b h) -> p b h", h=H)

                    # one-op one-hot build for the whole super-chunk
                    OT = otp.tile([128, nb * 128], bf16, tag="OT")
                    OT3 = OT[:].rearrange("p (b n) -> p b n", n=128)
                    d3 = dloc_sb[:, off_u:off_u + nb].to_broadcast([128, nb, 128])
                    i3 = iota_m.to_broadcast([128, nb, 128])
                    nc.vector.tensor_tensor(out=OT3, in0=d3, in1=i3,
                                            op=mybir.AluOpType.is_equal)

                    # fused V multiply for the whole super-chunk
                    V = vp.tile([128, nb * WH1], bf16, tag="V")
                    V3 = V[:].rearrange("p (b y) -> p b y", y=WH1)
                    V4 = V[:].rearrange("p (b h c) -> p b h c", h=H, c=C1)
                    G4 = G3[:, :, 0:WH1].rearrange("p b (h c) -> p b h c", c=C1)
                    w4 = w3.to_broadcast([128, nb, H, C1])
                    nc.vector.tensor_tensor(out=V4, in0=G4, in1=w4,
                                            op=mybir.AluOpType.mult)

                    # per bucket: scatter matmuls + self-loop fold + normalize
                    for bb in range(nbk):
                        bucket = b0 + bb
                        # self-loop contribution
                        ts = sf.tile([128, PAY], bf16, tag="ts")
                        if bucket < NBUCK_A:
                            tsrc = tbl_locA[bucket * 128:bucket * 128 + 128, 0:PAY]
                        else:
                            r0 = bucket * 128 - ROWS_A
                            rows = min(128, ROWS_B - r0)
                            tsrc = tbl_locB[r0:r0 + rows, 0:PAY]
                        nc.sync.dma_start(out=ts[:tsrc.shape[0], :], in_=tsrc)
                        es_s = sf.tile([128, H], f32, tag="es_s")
                        nc.vector.tensor_tensor(
                            out=es_s[:], in0=ts[:, WH1:PAY],
                            in1=er_full3[:, bucket, :], op=mybir.AluOpType.add)
                        ws_s = sf.tile([128, H], bf16, tag="ws_s")
                        ws1 = sf.tile([128, H], bf16, tag="ws1")
                        nc.scalar.activation(
                            out=ws1[:], in_=es_s[:],
                            func=mybir.ActivationFunctionType.Exp)
                        ws2 = sf.tile([128, H], bf16, tag="ws2")
                        nc.scalar.activation(
                            out=ws2[:], in_=es_s[:], scale=NEG,
                            func=mybir.ActivationFunctionType.Exp)
                        nc.vector.tensor_tensor(
                            out=ws_s[:], in0=ws1[:], in1=ws2[:],
                            op=mybir.AluOpType.max)
                        vs = sf.tile([128, WH1], bf16, tag="vs")
                        vs4 = vs[:].rearrange("p (h c) -> p h c", c=C1)
                        ts4 = ts[:, 0:WH1].rearrange("p (h c) -> p h c", c=C1)
                        nc.vector.tensor_tensor(
                            out=vs4, in0=ts4,
                            in1=ws_s[:].to_broadcast([128, H, C1]),
                            op=mybir.AluOpType.mult)

                        blks = ([bb * capA + j for j in range(capA)] +
                                [nbA + bb * capB + j for j in range(capB)])
                        ps = ps2p.tile([128, WH1], f32)
                        for i, blk in enumerate(blks):
                            nc.tensor.matmul(
                                out=ps[:], lhsT=OT3[:, blk, :], rhs=V3[:, blk, :],
                                start=(i == 0), stop=False)
                        nc.tensor.matmul(
                            out=ps[:], lhsT=ident[:], rhs=vs[:],
                            start=False, stop=True)

                        ps4 = ps[:].rearrange("p (h c) -> p h c", c=C1)
                        # den >= w_self > 0, so EPS=1e-16 is negligible
                        rec = np_.tile([128, H], f32, tag="rec")
                        nc.vector.reciprocal(rec[:], ps4[:, :, C])
                        ot = np_.tile([128, HC], bf16, tag="ot")
                        ot3 = ot[:].rearrange("p (h c) -> p h c", c=C)
                        r3 = rec[:].to_broadcast([128, H, C])
                        nc.vector.tensor_tensor(out=ot3, in0=ps4[:, :, 0:C],
                                                in1=r3, op=mybir.AluOpType.mult)
                        rows = min(128, NPC - bucket * 128)
                        nc.sync.dma_start(
                            out=out_ext[bucket * 128:bucket * 128 + rows, :],
                            in_=ot[:rows, :])

                LAG = 3
                for sc in range(n_sc):
                    emit_A(sc)
                    if sc >= LAG:
                        emit_B(sc - LAG)
                        emit_compute(sc - LAG)
                for sc in range(max(0, n_sc - LAG), n_sc):
                    emit_B(sc)
                    emit_compute(sc)

    _split_excess_waits(nc)
    _move_reload_after_collectives(nc)
    lower_extended_insts(nc)
    return nc


def kernel(**inputs):
    x = np.asarray(inputs["x"], np.float32)
    edge_index = np.asarray(inputs["edge_index"])
    W = np.asarray(inputs["W"], np.float32)
    a_left = np.asarray(inputs["a_left"], np.float32)
    a_right = np.asarray(inputs["a_right"], np.float32)

    (wtb, idxA, idxB, dloc_u, dlocT, xT, iota, iotaP, iotaPb,
     capA, capB) = _host_prep(x, edge_index, W, a_left, a_right)
    nc = _build_program(capA, capB)

    in_maps = []
    for c in range(NC):
        in_maps.append({
            "xT": np.ascontiguousarray(xT[c]),
            "wtb": wtb,
            "idxA": np.ascontiguousarray(idxA[c]),
            "idxB": np.ascontiguousarray(idxB[c]),
            "dloc": np.ascontiguousarray(dloc_u[c]),
            "dlocT": np.ascontiguousarray(dlocT[c]),
            "iota": iota,
            "iotaP": iotaP,
            "iotaPb": iotaPb,
        })

    res = run_bass_kernel_spmd(nc, in_maps, core_ids=list(range(NC)))
    out = np.concatenate([np.asarray(res.results[c]["out"]).astype(np.float32)
                          for c in range(NC)], axis=0)
    return out



# revision 10
# speedup vs baseline: 1.5353x; 1.0302x over previous
"""GAT layer on 8 Trainium2 NeuronCores (Bass/Tile), edge-parallel dst-sharded.

Self-contained. Host preprocesses the graph (dst-shard, bucket sort with
uniform caps, A/B split of each bucket's edges by source half for int16
gather indices; self-loops are NOT materialized as edges). Device program:

  phase 1: per 128-node tile, matmul x @ [W.T | a-folded] producing rows
    [Wh1(264) | el(8) | er(8)]; Wh1 interleaves a constant 1.0 after each
    head's 32 channels so a single multiply by w=exp(leaky(e)) yields both
    the weighted message AND the softmax-denominator column. Rows go to two
    local half-tables (stride 384 for dma_gather's 256B-stride rule) and a
    compact local er table.
  AllGather x2: half tables -> tblA/tblB on every core (AG1 fires once the
    first 25 buckets are written and overlaps the rest of phase 1).
  er expansion on the TENSOR engine (no DMA gather): per bucket, a
    transposed one-hot OTT (built from host-shipped per-slot dst indices)
    times the bucket's er rows gives per-edge-slot er for every block -
    packed into one PSUM tile per bucket and copied out. Runs during the
    AllGather window.
  phase 2 per super-chunk: two dma_gather calls (A/B) fetch table rows by
    src; score ops (add on DVE, leaky+exp on the scalar engine); one-op
    one-hot build; per-block V multiplies; per bucket capA+capB one-hot
    scatter matmuls accumulated in PSUM, then a final identity-matmul adds
    the analytically-computed self-loop contribution, and the bucket is
    normalized by the gathered denominator column and written out.

The SWDGE (GpSimd) descriptor path only carries the unavoidable per-edge
table gather; everything index-like that is bucket-local (er by dst, the
self loops) rides the tensor engine instead.
"""
import sys

for _p in ("/opt/trn_rl_repo",):
    if _p not in sys.path:
        sys.path.insert(0, _p)

import numpy as np
import ml_dtypes

import concourse.bass as bass
import concourse.tile as tile
from concourse import mybir, library_config
from concourse.bass_utils import run_bass_kernel_spmd
from concourse.ap_utils import ap_is_contiguous
from concourse.library_overlay import lower_extended_insts

BF16 = ml_dtypes.bfloat16

N = 50000
E = 800000
IN = 256
H = 8
C = 32
C1 = C + 1            # 33: [Wh_h(32) | 1]
HC = H * C            # 256
WH1 = H * C1          # 264
NC = 8
NPC = N // NC         # 6250 nodes per core
BUCKET = 128
NBUCK = (NPC + BUCKET - 1) // BUCKET   # 49
XT_PAD = NBUCK * 128                   # 6272
PAY = WH1 + H         # 272: gather payload [Wh1(264) | el(8)]
P1COLS = PAY + H      # 280: phase-1 matmul out [Wh1 | el | er]
TROW = 384            # table row stride (256B multiple)
GELEM = 272           # gathered elements per row
NEG = 0.2
EPS = 1e-16
SC_BUCKETS = 1        # buckets per gather super-chunk
NBUCK_A = 25          # buckets 0..24 -> table half A
ROWS_A = NBUCK_A * BUCKET          # 3200
ROWS_B = NPC - ROWS_A              # 3050
USE_PREP_TRIGGER = False
USE_ACT_LRELU = False
NQ = 4

_waitfix_ctr = [0]
_reg_cache = {}


def _split_excess_waits(nc, max_waits=1):
    # walrus in this container caps sync waits per instruction at 1; hoist
    # excess onto same-engine NoOps.
    n_fixed = 0
    for fn in nc.m.functions:
        for bb in fn.blocks:
            insts = bb.instructions
            out = []
            for ins in insts:
                si = ins.sync_info
                waits = list(si.on_wait) if si is not None and si.on_wait else []
                if len(waits) > max_waits:
                    keep = waits[-max_waits:]
                    extra = waits[:-max_waits]
                    for i in range(0, len(extra), max_waits):
                        grp = extra[i:i + max_waits]
                        _waitfix_ctr[0] += 1
                        nop = mybir.InstNoOp(
                            name=f"I-waitfix-{_waitfix_ctr[0]}", ins=[], outs=[])
                        nop.engine = ins.engine
                        nop.sync_info = mybir.SyncInfo(on_wait=grp, on_update=[])
                        nc.register_instruction(nop)
                        out.append(nop)
                    si.on_wait = keep
                    n_fixed += 1
                out.append(ins)
            if len(out) != len(insts):
                bb.instructions = out
    return n_fixed


def _move_reload_after_collectives(nc):
    """The tile scheduler floats the dependency-less library-reload pseudo to
    the top of the program; keep it after the last collective trigger."""
    from concourse import bass_isa
    for fn in nc.m.functions:
        for bb in fn.blocks:
            insts = bb.instructions
            reload_idx = [i for i, ins in enumerate(insts)
                          if isinstance(ins, bass_isa.InstPseudoReloadLibraryIndex)]
            coll_idx = [i for i, ins in enumerate(insts)
                        if isinstance(ins, mybir.InstCollectiveCompute)]
            if not reload_idx or not coll_idx:
                continue
            assert len(reload_idx) == 1
            r = reload_idx[0]
            last_c = max(coll_idx)
            if r > last_c:
                continue
            ins = insts.pop(r)
            insts.insert(last_c, ins)
            bb.instructions = insts
    return nc


def _dma_gather_raw(eng, out_ap, in_ap, idxs_ap, num_idxs, elem_size, elem_step,
                    sem=None, queue_num=0):
    """bass.dma_gather without the elem_size_bytes%256 assert (non-transpose,
    DRAM source, 256B-multiple row stride). sem!=None -> prepare_only."""
    assert idxs_ap.dtype == mybir.dt.int16
    assert in_ap.dtype == out_ap.dtype
    assert ap_is_contiguous(out_ap.ap[1:])
    assert ap_is_contiguous(idxs_ap.ap[1:])
    assert in_ap.ap[0][0] == elem_step
    stride_bytes = elem_step * mybir.dt.size(in_ap.dtype)
    stride_bytes_256 = stride_bytes // 256
    assert stride_bytes_256 * 256 == stride_bytes and stride_bytes_256 < 256
    _in_ap = eng.lower_ap_dma(in_ap, for_custom_bir_dma=True)
    _idxs_ap = eng.lower_ap(idxs_ap)
    _out_ap = eng.lower_ap(out_ap)
    key = (id(eng.bass), num_idxs)
    if key not in _reg_cache:
        _reg_cache[key] = eng.to_reg(num_idxs)
    inst = eng.add_instruction(
        mybir.InstDMAGatherAnt(
            name=eng.bass.get_next_instruction_name(),
            ins=[*_in_ap, _idxs_ap, eng.lower_val_access(_reg_cache[key])],
            outs=[_out_ap],
            transpose=False,
            num_idxs=num_idxs,
            elem_size=elem_size,
            stride_bytes_256=stride_bytes_256,
            gen_mode=int(sem is not None),
            single_packet=False,
            queue_num=queue_num,
            sbuf_tokens_per_rank=0,
            sbuf_free_dim_per_rank=0,
            sbuf_free_dim_pad_per_rank=0,
            sbuf_byte_offset=0,
        )
    )
    if sem is not None:
        inst.then_inc(sem, 16)
        return eng._track_prepare_only(inst, queue_num)
    return inst


def _wrap16(vals):
    """Edge-slot int16 index array -> dma_gather layout [128, n/16]."""
    n = len(vals)
    assert n % 16 == 0
    w = np.asarray(vals, np.int16).reshape(n // 16, 16).T
    return np.tile(w, (8, 1))


def _host_prep(x, edge_index, W, a_left, a_right):
    src = np.asarray(edge_index[0], np.int64)
    dst = np.asarray(edge_index[1], np.int64)

    # fold attention vectors through W:  [el|er] = x @ (W.T @ A)
    A = np.zeros((HC, 2 * H), np.float32)
    for h in range(H):
        A[h * C:(h + 1) * C, h] = a_left[h]
        A[h * C:(h + 1) * C, H + h] = a_right[h]
    B = (W.T.astype(np.float64) @ A.astype(np.float64)).astype(np.float32)
    wtb = np.zeros((IN, P1COLS), np.float32)
    for h in range(H):
        wtb[:, h * C1:h * C1 + C] = W.T[:, h * C:(h + 1) * C]
    wtb[:, WH1:WH1 + H] = B[:, :H]
    wtb[:, WH1 + H:] = B[:, H:]
    wtb = wtb.astype(BF16)

    core = dst // NPC
    r_src = src % NPC
    c_src = src // NPC
    is_a = r_src < ROWS_A
    gidx = np.where(is_a, c_src * ROWS_A + r_src,
                    c_src * ROWS_B + (r_src - ROWS_A)).astype(np.int64)

    capA = capB = 0
    lists = {}
    for c in range(NC):
        m = core == c
        s_c, d_c, g_c, a_c = src[m], dst[m], gidx[m], is_a[m]
        dl = d_c - c * NPC
        b_c = dl // BUCKET
        order = np.lexsort((s_c, b_c))
        s_c, dl, b_c, g_c, a_c = (s_c[order], dl[order], b_c[order],
                                  g_c[order], a_c[order])
        cnt = np.bincount(b_c, minlength=NBUCK)
        starts = np.concatenate([[0], np.cumsum(cnt)[:-1]])
        for b in range(NBUCK):
            sl = slice(starts[b], starts[b] + cnt[b])
            aa = a_c[sl]
            lists[(c, b)] = (g_c[sl][aa], dl[sl][aa] - b * BUCKET,
                             g_c[sl][~aa], dl[sl][~aa] - b * BUCKET)
            capA = max(capA, (int(aa.sum()) + 127) // 128)
            capB = max(capB, (int((~aa).sum()) + 127) // 128)

    nblkb = capA + capB
    nblk = NBUCK * nblkb
    nblkA = NBUCK * capA
    nblkB = NBUCK * capB
    n_sc = (NBUCK + SC_BUCKETS - 1) // SC_BUCKETS

    idxA = np.zeros((NC, 128, nblkA * 8), np.int16)
    idxB = np.zeros((NC, 128, nblkB * 8), np.int16)
    dloc_u = np.full((NC, 128, nblk), 200.0, BF16)
    dlocT = np.full((NC, 128, nblk * 128), -1, np.int8)
    xT = np.zeros((NC, IN, XT_PAD), BF16)

    for c in range(NC):
        iA = np.zeros(nblkA * 128, np.int64)
        iB = np.zeros(nblkB * 128, np.int64)
        dA = np.full((nblkA, 128), -1, np.int64)
        dB = np.full((nblkB, 128), -1, np.int64)
        for b in range(NBUCK):
            gA, dlA, gB, dlB = lists[(c, b)]
            oa = b * capA * 128
            ob = b * capB * 128
            iA[oa:oa + len(gA)] = gA
            iB[ob:ob + len(gB)] = gB
            fa = dA[b * capA:(b + 1) * capA].reshape(-1)
            fa[:len(dlA)] = dlA
            fb = dB[b * capB:(b + 1) * capB].reshape(-1)
            fb[:len(dlB)] = dlB
        idxA[c] = _wrap16(iA)
        idxB[c] = _wrap16(iB)
        # slot k of block j = (partition k%128); dA rows are flat slot runs
        dA = dA.reshape(nblkA, 128)
        dB = dB.reshape(nblkB, 128)
        # dloc_u: SC-major order [per SC: A-blocks | B-blocks], [128, nblk]
        off = 0
        for sc in range(n_sc):
            b0 = sc * SC_BUCKETS
            bs = range(b0, min(b0 + SC_BUCKETS, NBUCK))
            for b in bs:
                blk = dA[b * capA:(b + 1) * capA]     # [capA, 128]
                v = np.where(blk < 0, 200.0, blk).astype(np.float32)
                dloc_u[c, :, off:off + capA] = v.T.astype(BF16)
                off += capA
            for b in bs:
                blk = dB[b * capB:(b + 1) * capB]
                v = np.where(blk < 0, 200.0, blk).astype(np.float32)
                dloc_u[c, :, off:off + capB] = v.T.astype(BF16)
                off += capB
        # dlocT: BUCKET-major order [per bucket: A-blocks | B-blocks],
        # transposed and replicated: [128(any), (b*nblkb + j)*128 + p]
        for b in range(NBUCK):
            base = b * nblkb * 128
            rows = np.concatenate(
                [dA[b * capA:(b + 1) * capA], dB[b * capB:(b + 1) * capB]],
                axis=0)                                # [nblkb, 128]
            dlocT[c, :, base:base + nblkb * 128] = np.broadcast_to(
                rows.reshape(-1).astype(np.int8), (128, nblkb * 128))

        xs = x[c * NPC:(c + 1) * NPC].astype(BF16)
        xT[c, :, :NPC] = xs.T

    iota = np.tile(np.arange(128, dtype=np.float32)[None, :], (128, 1)).astype(BF16)
    iotaP = np.arange(128, dtype=np.int8).reshape(128, 1)
    iotaPb = np.arange(128, dtype=np.float32).reshape(128, 1).astype(BF16)
    return (wtb, idxA, idxB, dloc_u, dlocT, xT, iota, iotaP, iotaPb,
            capA, capB)


def _build_program(capA, capB):
    nblkb = capA + capB
    nblk = NBUCK * nblkb
    nblkA = NBUCK * capA
    nblkB = NBUCK * capB
    f32 = mybir.dt.float32
    bf16 = mybir.dt.bfloat16
    i16 = mybir.dt.int16
    i8 = mybir.dt.int8

    nc = bass.Bass(trn_type="TRN2", num_devices=NC, num_swdge_queues=NQ)
    xT_in = nc.declare_dram_parameter("xT", [IN, XT_PAD], bf16, isOutput=False)
    wtb_in = nc.declare_dram_parameter("wtb", [IN, P1COLS], bf16, isOutput=False)
    idxA_in = nc.declare_dram_parameter("idxA", [128, nblkA * 8], i16, isOutput=False)
    idxB_in = nc.declare_dram_parameter("idxB", [128, nblkB * 8], i16, isOutput=False)
    dloc_in = nc.declare_dram_parameter("dloc", [128, nblk], bf16, isOutput=False)
    dlocT_in = nc.declare_dram_parameter("dlocT", [128, nblk * 128], i8, isOutput=False)
    iota_in = nc.declare_dram_parameter("iota", [128, 128], bf16, isOutput=False)
    iotaP_in = nc.declare_dram_parameter("iotaP", [128, 1], i8, isOutput=False)
    iotaPb_in = nc.declare_dram_parameter("iotaPb", [128, 1], bf16, isOutput=False)
    out_ext = nc.declare_dram_parameter("out", [NPC, HC], bf16, isOutput=True)

    tbl_locA = nc.dram_tensor("tbl_locA", [ROWS_A, TROW], bf16)
    tbl_locB = nc.dram_tensor("tbl_locB", [ROWS_B, TROW], bf16)
    tblA = nc.dram_tensor("tblA", [NC * ROWS_A, TROW], bf16, addr_space="Shared")
    tblB = nc.dram_tensor("tblB", [NC * ROWS_B, TROW], bf16, addr_space="Shared")

    with tile.TileContext(nc) as tc:
        with tc.tile_pool(name="cst", bufs=1) as cst:
            # er for all buckets, bucket-partition layout [p, b, h]; filled
            # directly from phase-1 PSUM (dead tail rows produce er=0)
            er_full = cst.tile([128, NBUCK * H], bf16)
            er_full3 = er_full[:].rearrange("p (b h) -> p b h", h=H)
            # ---------------- phase 1: Wh1 / el / er ----------------
            with tc.tile_pool(name="p1w", bufs=1) as p1w, \
                 tc.tile_pool(name="p1", bufs=3) as p1, \
                 tc.tile_pool(name="ps1", bufs=2, space="PSUM") as ps1:
                xts = []
                wtbs = []
                for k in range(2):
                    t = p1w.tile([128, XT_PAD], bf16, tag=f"xt{k}")
                    nc.sync.dma_start(out=t[:], in_=xT_in[k * 128:(k + 1) * 128, :])
                    xts.append(t)
                    u = p1w.tile([128, P1COLS], bf16, tag=f"wtb{k}")
                    nc.sync.dma_start(out=u[:], in_=wtb_in[k * 128:(k + 1) * 128, :])
                    wtbs.append(u)
                groups = []
                for b0s, b1s in ((0, NBUCK_A), (NBUCK_A, NBUCK - 1)):
                    b = b0s
                    while b < b1s:
                        g = min(4, b1s - b)
                        groups.append((b, g))
                        b += g
                groups.append((NBUCK - 1, 1))
                BANK = 512          # f32 elems per PSUM bank
                for b0, g in groups:
                    ps = ps1.tile([128, 4 * BANK], f32)
                    ps3 = ps[:].rearrange("p (g y) -> p g y", y=BANK)
                    for i in range(g):
                        tn = b0 + i
                        for k in range(2):
                            nc.tensor.matmul(
                                out=ps3[:, i, 0:P1COLS],
                                lhsT=xts[k][:, tn * 128:(tn + 1) * 128],
                                rhs=wtbs[k][:],
                                start=(k == 0), stop=(k == 1),
                            )
                    sb = p1.tile([128, 4 * P1COLS], bf16)
                    sb3g = sb[:, 0:g * P1COLS].rearrange("p (g y) -> p g y", y=P1COLS)
                    nc.scalar.activation(out=sb3g,
                                         in_=ps3[:, 0:g, 0:P1COLS],
                                         func=mybir.ActivationFunctionType.Copy)
                    sb4 = sb[:, 0:g * P1COLS].rearrange(
                        "p (g y) -> p g y", y=P1COLS)[:, :, 0:WH1].rearrange(
                        "p g (h c) -> p g h c", c=C1)
                    nc.vector.memset(sb4[:, :, :, C:C1], 1.0)
                    nc.scalar.activation(out=er_full3[:, b0:b0 + g, :],
                                         in_=ps3[:, 0:g, PAY:P1COLS],
                                         func=mybir.ActivationFunctionType.Copy)
                    sb3 = sb[:, 0:g * P1COLS].rearrange("p (g y) -> p g y", y=P1COLS)
                    if b0 + g <= NBUCK_A:
                        dst = tbl_locA[b0 * 128:(b0 + g) * 128, 0:PAY]
                    else:
                        r0 = b0 * 128 - ROWS_A
                        rows = min(g * 128, ROWS_B - r0)
                        dst = tbl_locB[r0:r0 + rows, 0:PAY]
                    if dst.shape[0] == g * 128:
                        dst3 = dst.rearrange("(g p) y -> p g y", p=128)
                        nc.sync.dma_start(out=dst3, in_=sb3[:, :, 0:PAY])
                    else:
                        nc.sync.dma_start(out=dst, in_=sb[:dst.shape[0], 0:PAY])

            # ---------------- all-gather both half tables ----------------
            nc.gpsimd.collective_compute(
                "AllGather", mybir.AluOpType.bypass,
                replica_groups=[list(range(NC))],
                ins=[tbl_locA[:].opt()], outs=[tblA[:].opt()],
            )
            nc.gpsimd.collective_compute(
                "AllGather", mybir.AluOpType.bypass,
                replica_groups=[list(range(NC))],
                ins=[tbl_locB[:].opt()], outs=[tblB[:].opt()],
            )
            nc.gpsimd.load_library(library_config.mlp)

            iota_b = cst.tile([128, 128], bf16)
            nc.sync.dma_start(out=iota_b[:], in_=iota_in[:, :])
            iota_m = iota_b[:].rearrange("p (b n) -> p b n", b=1)
            iotaP_sb = cst.tile([128, 1], i8)
            nc.sync.dma_start(out=iotaP_sb[:], in_=iotaP_in[:, :])
            iotaP_m = iotaP_sb[:].rearrange("p (b n) -> p b n", b=1)
            iotaPb_sb = cst.tile([128, 1], bf16)
            nc.sync.dma_start(out=iotaPb_sb[:], in_=iotaPb_in[:, :])
            ident = cst.tile([128, 128], bf16)
            nc.vector.tensor_tensor(
                out=ident[:], in0=iotaPb_sb[:].to_broadcast([128, 128]),
                in1=iota_b[:], op=mybir.AluOpType.is_equal)

            idxA_sb = cst.tile([128, nblkA * 8], i16)
            nc.sync.dma_start(out=idxA_sb[:], in_=idxA_in[:, :])
            idxB_sb = cst.tile([128, nblkB * 8], i16)
            nc.sync.dma_start(out=idxB_sb[:], in_=idxB_in[:, :])
            dloc_sb = cst.tile([128, nblk], bf16)
            nc.sync.dma_start(out=dloc_sb[:], in_=dloc_in[:, :])
            # ---------------- er expansion on PE (overlaps AllGather) ------
            er_e = cst.tile([128, nblk * 8], bf16)   # SC-major slot order
            with tc.tile_pool(name="otq", bufs=2) as otq, \
                 tc.tile_pool(name="dtq", bufs=2) as dtq, \
                 tc.tile_pool(name="pse", bufs=2, space="PSUM") as pse, \
                 tc.tile_pool(name="gp", bufs=8) as gp, \
                 tc.tile_pool(name="vp", bufs=2) as vp, \
                 tc.tile_pool(name="otp", bufs=3) as otp, \
                 tc.tile_pool(name="sp", bufs=2) as sp, \
                 tc.tile_pool(name="sf", bufs=3) as sf, \
                 tc.tile_pool(name="np_", bufs=3) as np_, \
                 tc.tile_pool(name="ps2", bufs=4, space="PSUM") as ps2p:
                n_sc = (NBUCK + SC_BUCKETS - 1) // SC_BUCKETS
                for b in range(NBUCK):
                    dT = dtq.tile([128, nblkb * 128], i8, tag="dT")
                    nc.sync.dma_start(
                        out=dT[:],
                        in_=dlocT_in[:, b * nblkb * 128:(b + 1) * nblkb * 128])
                    dT3 = dT[:].rearrange("p (j n) -> p j n", n=128)
                    OTT = otq.tile([128, nblkb * 128], bf16, tag="OTT")
                    OTT3 = OTT[:].rearrange("p (j n) -> p j n", n=128)
                    nc.vector.tensor_tensor(
                        out=OTT3, in0=iotaP_m.to_broadcast([128, nblkb, 128]),
                        in1=dT3, op=mybir.AluOpType.is_equal)
                    pe = pse.tile([128, nblkb * H], f32)
                    pe3 = pe[:].rearrange("p (j h) -> p j h", h=H)
                    for j in range(nblkb):
                        nc.tensor.matmul(
                            out=pe3[:, j, :], lhsT=OTT3[:, j, :],
                            rhs=er_full3[:, b, :], start=True, stop=True)
                    # copy into er_e at SC-major positions (A-run | B-run)
                    sc = b // SC_BUCKETS
                    b0 = sc * SC_BUCKETS
                    nbk = min(SC_BUCKETS, NBUCK - b0)
                    off_sc = b0 * nblkb
                    offA = (off_sc + (b - b0) * capA) * 8
                    offB = (off_sc + nbk * capA + (b - b0) * capB) * 8
                    nc.scalar.activation(
                        out=er_e[:, offA:offA + capA * 8],
                        in_=pe[:, 0:capA * 8],
                        func=mybir.ActivationFunctionType.Copy)
                    nc.scalar.activation(
                        out=er_e[:, offB:offB + capB * 8],
                        in_=pe[:, capA * 8:nblkb * 8],
                        func=mybir.ActivationFunctionType.Copy)

                # ---------------- phase 2: gather / score / scatter --------
                pend = {}
                qctr = [0]

                def next_q():
                    q = qctr[0] % NQ
                    qctr[0] += 1
                    return q

                def emit_A(sc):
                    b0 = sc * SC_BUCKETS
                    nbk = min(SC_BUCKETS, NBUCK - b0)
                    nbA = nbk * capA
                    nbB = nbk * capB
                    nb = nbA + nbB
                    G = gp.tile([128, nb * GELEM], bf16, tag="G")
                    G3 = G[:].rearrange("p (b y) -> p b y", y=GELEM)
                    a0 = b0 * capA * 8
                    _dma_gather_raw(
                        nc.gpsimd, G3[:, 0:nbA, :], tblA[:],
                        idxA_sb[:, a0:a0 + nbA * 8], nbA * 128,
                        GELEM, TROW, queue_num=next_q())
                    pend[sc] = (G3, b0, nbk, nbA, nbB, nb)

                def emit_B(sc):
                    G3, b0, nbk, nbA, nbB, nb = pend[sc]
                    bb0 = b0 * capB * 8
                    _dma_gather_raw(
                        nc.gpsimd, G3[:, nbA:nb, :], tblB[:],
                        idxB_sb[:, bb0:bb0 + nbB * 8], nbB * 128,
                        GELEM, TROW, queue_num=next_q())

                def emit_compute(sc):
                    G3, b0, nbk, nbA, nbB, nb = pend.pop(sc)
                    off_u = b0 * nblkb
                    # scores: e = el + er ; leaky ; exp
                    e_t = sp.tile([128, nb * H], f32, tag="e")
                    e3 = e_t[:].rearrange("p (b h) -> p b h", h=H)
                    er_sc3 = er_e[:, off_u * 8:(off_u + nb) * 8].rearrange(
                        "p (b h) -> p b h", h=H)
                    nc.vector.tensor_tensor(
                        out=e3, in0=G3[:, :, WH1:PAY], in1=er_sc3,
                        op=mybir.AluOpType.add)
                    # w = exp(leaky(e)) = max(exp(e), exp(NEG*e)) (exp monotone)
                    w_t = sp.tile([128, nb * H], bf16, tag="w")
                    w1_t = sp.tile([128, nb * H], bf16, tag="w1")
                    nc.scalar.activation(
                        out=w1_t[:], in_=e_t[:],
                        func=mybir.ActivationFunctionType.Exp)
                    w2_t = sp.tile([128, nb * H], bf16, tag="w2")
                    nc.scalar.activation(
                        out=w2_t[:], in_=e_t[:], scale=NEG,
                        func=mybir.ActivationFunctionType.Exp)
                    nc.vector.tensor_tensor(
                        out=w_t[:], in0=w1_t[:], in1=w2_t[:],
                        op=mybir.AluOpType.max)
                    w3 = w_t[:].rearrange("p (b h) -> p b h", h=H)

                    # one-op one-hot build for the whole super-chunk
                    OT = otp.tile([128, nb * 128], bf16, tag="OT")
                    OT3 = OT[:].rearrange("p (b n) -> p b n", n=128)
                    d3 = dloc_sb[:, off_u:off_u + nb].to_broadcast([128, nb, 128])
                    i3 = iota_m.to_broadcast([128, nb, 128])
                    nc.vector.tensor_tensor(out=OT3, in0=d3, in1=i3,
                                            op=mybir.AluOpType.is_equal)

                    # fused V multiply for the whole super-chunk
                    V = vp.tile([128, nb * WH1], bf16, tag="V")
                    V3 = V[:].rearrange("p (b y) -> p b y", y=WH1)
                    V4 = V[:].rearrange("p (b h c) -> p b h c", h=H, c=C1)
                    G4 = G3[:, :, 0:WH1].rearrange("p b (h c) -> p b h c", c=C1)
                    w4 = w3.to_broadcast([128, nb, H, C1])
                    nc.vector.tensor_tensor(out=V4, in0=G4, in1=w4,
                                            op=mybir.AluOpType.mult)

                    # per bucket: scatter matmuls + self-loop fold + normalize
                    for bb in range(nbk):
                        bucket = b0 + bb
                        # self-loop contribution
                        ts = sf.tile([128, PAY], bf16, tag="ts")
                        if bucket < NBUCK_A:
                            tsrc = tbl_locA[bucket * 128:bucket * 128 + 128, 0:PAY]
                        else:
                            r0 = bucket * 128 - ROWS_A
                            rows = min(128, ROWS_B - r0)
                            tsrc = tbl_locB[r0:r0 + rows, 0:PAY]
                        nc.sync.dma_start(out=ts[:tsrc.shape[0], :], in_=tsrc)
                        es_s = sf.tile([128, H], f32, tag="es_s")
                        nc.vector.tensor_tensor(
                            out=es_s[:], in0=ts[:, WH1:PAY],
                            in1=er_full3[:, bucket, :], op=mybir.AluOpType.add)
                        ws_s = sf.tile([128, H], bf16, tag="ws_s")
                        ws1 = sf.tile([128, H], bf16, tag="ws1")
                        nc.scalar.activation(
                            out=ws1[:], in_=es_s[:],
                            func=mybir.ActivationFunctionType.Exp)
                        ws2 = sf.tile([128, H], bf16, tag="ws2")
                        nc.scalar.activation(
                            out=ws2[:], in_=es_s[:], scale=NEG,
                            func=mybir.ActivationFunctionType.Exp)
                        nc.vector.tensor_tensor(
                            out=ws_s[:], in0=ws1[:], in1=ws2[:],
                            op=mybir.AluOpType.max)
                        vs = sf.tile([128, WH1], bf16, tag="vs")
                        vs4 = vs[:].rearrange("p (h c) -> p h c", c=C1)
                        ts4 = ts[:, 0:WH1].rearrange("p (h c) -> p h c", c=C1)
                        nc.vector.tensor_tensor(
                            out=vs4, in0=ts4,
                            in1=ws_s[:].to_broadcast([128, H, C1]),
                            op=mybir.AluOpType.mult)

                        blks = ([bb * capA + j for j in range(capA)] +
                                [nbA + bb * capB + j for j in range(capB)])
                        ps = ps2p.tile([128, WH1], f32)
                        for i, blk in enumerate(blks):
                            nc.tensor.matmul(
                                out=ps[:], lhsT=OT3[:, blk, :], rhs=V3[:, blk, :],
                                start=(i == 0), stop=False)
                        nc.tensor.matmul(
                            out=ps[:], lhsT=ident[:], rhs=vs[:],
                            start=False, stop=True)

                        ps4 = ps[:].rearrange("p (h c) -> p h c", c=C1)
                        # den >= w_self > 0, so EPS=1e-16 is negligible
                        rec = np_.tile([128, H], f32, tag="rec")
                        nc.vector.reciprocal(rec[:], ps4[:, :, C])
                        ot = np_.tile([128, HC], bf16, tag="ot")
                        ot3 = ot[:].rearrange("p (h c) -> p h c", c=C)
                        r3 = rec[:].to_broadcast([128, H, C])
                        nc.vector.tensor_tensor(out=ot3, in0=ps4[:, :, 0:C],
                                                in1=r3, op=mybir.AluOpType.mult)
                        rows = min(128, NPC - bucket * 128)
                        nc.sync.dma_start(
                            out=out_ext[bucket * 128:bucket * 128 + rows, :],
                            in_=ot[:rows, :])

                # A-gathers run LA buckets ahead so they fill the AG-B
                # window; B+compute trail together (B waits on AG-B).
                LA = 7
                for t in range(n_sc + LA):
                    if t >= LA:
                        emit_B(t - LA)
                        emit_compute(t - LA)
                    if t < n_sc:
                        emit_A(t)

    _split_excess_waits(nc)
    _move_reload_after_collectives(nc)
    lower_extended_insts(nc)
    return nc


def kernel(**inputs):
    x = np.asarray(inputs["x"], np.float32)
    edge_index = np.asarray(inputs["edge_index"])
    W = np.asarray(inputs["W"], np.float32)
    a_left = np.asarray(inputs["a_left"], np.float32)
    a_right = np.asarray(inputs["a_right"], np.float32)

    (wtb, idxA, idxB, dloc_u, dlocT, xT, iota, iotaP, iotaPb,
     capA, capB) = _host_prep(x, edge_index, W, a_left, a_right)
    nc = _build_program(capA, capB)

    in_maps = []
    for c in range(NC):
        in_maps.append({
            "xT": np.ascontiguousarray(xT[c]),
            "wtb": wtb,
            "idxA": np.ascontiguousarray(idxA[c]),
            "idxB": np.ascontiguousarray(idxB[c]),
            "dloc": np.ascontiguousarray(dloc_u[c]),
            "dlocT": np.ascontiguousarray(dlocT[c]),
            "iota": iota,
            "iotaP": iotaP,
            "iotaPb": iotaPb,
        })

    res = run_bass_kernel_spmd(nc, in_maps, core_ids=list(range(NC)))
    out = np.concatenate([np.asarray(res.results[c]["out"]).astype(np.float32)
                          for c in range(NC)], axis=0)
    return out



# revision 13
# speedup vs baseline: 1.5411x; 1.0038x over previous
"""GAT layer on 8 Trainium2 NeuronCores (Bass/Tile), edge-parallel dst-sharded.

Self-contained. Host preprocesses the graph (dst-shard, bucket sort with
uniform caps, A/B split of each bucket's edges by source half for int16
gather indices; self-loops are NOT materialized as edges). Device program:

  phase 1: per 128-node tile, matmul x @ [W.T | a-folded] producing rows
    [Wh1(264) | el(8) | er(8)]; Wh1 interleaves a constant 1.0 after each
    head's 32 channels so a single multiply by w=exp(leaky(e)) yields both
    the weighted message AND the softmax-denominator column. Rows go to two
    local half-tables (stride 384 for dma_gather's 256B-stride rule) and a
    compact local er table.
  AllGather x2: half tables -> tblA/tblB on every core (AG1 fires once the
    first 25 buckets are written and overlaps the rest of phase 1).
  er expansion on the TENSOR engine (no DMA gather): per bucket, a
    transposed one-hot OTT (built from host-shipped per-slot dst indices)
    times the bucket's er rows gives per-edge-slot er for every block -
    packed into one PSUM tile per bucket and copied out. Runs during the
    AllGather window.
  phase 2 per super-chunk: two dma_gather calls (A/B) fetch table rows by
    src; score ops (add on DVE, leaky+exp on the scalar engine); one-op
    one-hot build; per-block V multiplies; per bucket capA+capB one-hot
    scatter matmuls accumulated in PSUM, then a final identity-matmul adds
    the analytically-computed self-loop contribution, and the bucket is
    normalized by the gathered denominator column and written out.

The SWDGE (GpSimd) descriptor path only carries the unavoidable per-edge
table gather; everything index-like that is bucket-local (er by dst, the
self loops) rides the tensor engine instead.
"""
import sys

for _p in ("/opt/trn_rl_repo",):
    if _p not in sys.path:
        sys.path.insert(0, _p)

import numpy as np
import ml_dtypes

import concourse.bass as bass
import concourse.tile as tile
from concourse import mybir, library_config
from concourse.bass_utils import run_bass_kernel_spmd
from concourse.ap_utils import ap_is_contiguous
from concourse.library_overlay import lower_extended_insts

BF16 = ml_dtypes.bfloat16

N = 50000
E = 800000
IN = 256
H = 8
C = 32
C1 = C + 1            # 33: [Wh_h(32) | 1]
HC = H * C            # 256
WH1 = H * C1          # 264
NC = 8
NPC = N // NC         # 6250 nodes per core
BUCKET = 128
NBUCK = (NPC + BUCKET - 1) // BUCKET   # 49
XT_PAD = NBUCK * 128                   # 6272
PAY = WH1 + H         # 272: gather payload [Wh1(264) | el(8)]
P1COLS = PAY + H      # 280: phase-1 matmul out [Wh1 | el | er]
TROW = 384            # table row stride (256B multiple)
GELEM = 272           # gathered elements per row
NEG = 0.2
EPS = 1e-16
SC_BUCKETS = 1        # buckets per gather super-chunk
NBUCK_A = 25          # buckets 0..24 -> table half A
ROWS_A = NBUCK_A * BUCKET          # 3200
ROWS_B = NPC - ROWS_A              # 3050
USE_PREP_TRIGGER = False
USE_ACT_LRELU = False
NQ = 4

_waitfix_ctr = [0]
_reg_cache = {}


def _split_excess_waits(nc, max_waits=1):
    # walrus in this container caps sync waits per instruction at 1; hoist
    # excess onto same-engine NoOps.
    n_fixed = 0
    for fn in nc.m.functions:
        for bb in fn.blocks:
            insts = bb.instructions
            out = []
            for ins in insts:
                si = ins.sync_info
                waits = list(si.on_wait) if si is not None and si.on_wait else []
                if len(waits) > max_waits:
                    keep = waits[-max_waits:]
                    extra = waits[:-max_waits]
                    for i in range(0, len(extra), max_waits):
                        grp = extra[i:i + max_waits]
                        _waitfix_ctr[0] += 1
                        nop = mybir.InstNoOp(
                            name=f"I-waitfix-{_waitfix_ctr[0]}", ins=[], outs=[])
                        nop.engine = ins.engine
                        nop.sync_info = mybir.SyncInfo(on_wait=grp, on_update=[])
                        nc.register_instruction(nop)
                        out.append(nop)
                    si.on_wait = keep
                    n_fixed += 1
                out.append(ins)
            if len(out) != len(insts):
                bb.instructions = out
    return n_fixed


def _move_reload_after_collectives(nc):
    """The tile scheduler floats the dependency-less library-reload pseudo to
    the top of the program; keep it after the last collective trigger."""
    from concourse import bass_isa
    for fn in nc.m.functions:
        for bb in fn.blocks:
            insts = bb.instructions
            reload_idx = [i for i, ins in enumerate(insts)
                          if isinstance(ins, bass_isa.InstPseudoReloadLibraryIndex)]
            coll_idx = [i for i, ins in enumerate(insts)
                        if isinstance(ins, mybir.InstCollectiveCompute)]
            if not reload_idx or not coll_idx:
                continue
            assert len(reload_idx) == 1
            r = reload_idx[0]
            last_c = max(coll_idx)
            if r > last_c:
                continue
            ins = insts.pop(r)
            insts.insert(last_c, ins)
            bb.instructions = insts
    return nc


def _dma_gather_raw(eng, out_ap, in_ap, idxs_ap, num_idxs, elem_size, elem_step,
                    sem=None, queue_num=0):
    """bass.dma_gather without the elem_size_bytes%256 assert (non-transpose,
    DRAM source, 256B-multiple row stride). sem!=None -> prepare_only."""
    assert idxs_ap.dtype == mybir.dt.int16
    assert in_ap.dtype == out_ap.dtype
    assert ap_is_contiguous(out_ap.ap[1:])
    assert ap_is_contiguous(idxs_ap.ap[1:])
    assert in_ap.ap[0][0] == elem_step
    stride_bytes = elem_step * mybir.dt.size(in_ap.dtype)
    stride_bytes_256 = stride_bytes // 256
    assert stride_bytes_256 * 256 == stride_bytes and stride_bytes_256 < 256
    _in_ap = eng.lower_ap_dma(in_ap, for_custom_bir_dma=True)
    _idxs_ap = eng.lower_ap(idxs_ap)
    _out_ap = eng.lower_ap(out_ap)
    key = (id(eng.bass), num_idxs)
    if key not in _reg_cache:
        _reg_cache[key] = eng.to_reg(num_idxs)
    inst = eng.add_instruction(
        mybir.InstDMAGatherAnt(
            name=eng.bass.get_next_instruction_name(),
            ins=[*_in_ap, _idxs_ap, eng.lower_val_access(_reg_cache[key])],
            outs=[_out_ap],
            transpose=False,
            num_idxs=num_idxs,
            elem_size=elem_size,
            stride_bytes_256=stride_bytes_256,
            gen_mode=int(sem is not None),
            single_packet=False,
            queue_num=queue_num,
            sbuf_tokens_per_rank=0,
            sbuf_free_dim_per_rank=0,
            sbuf_free_dim_pad_per_rank=0,
            sbuf_byte_offset=0,
        )
    )
    if sem is not None:
        inst.then_inc(sem, 16)
        return eng._track_prepare_only(inst, queue_num)
    return inst


def _wrap16(vals):
    """Edge-slot int16 index array -> dma_gather layout [128, n/16]."""
    n = len(vals)
    assert n % 16 == 0
    w = np.asarray(vals, np.int16).reshape(n // 16, 16).T
    return np.tile(w, (8, 1))


def _host_prep(x, edge_index, W, a_left, a_right):
    src = np.asarray(edge_index[0], np.int64)
    dst = np.asarray(edge_index[1], np.int64)

    # fold attention vectors through W:  [el|er] = x @ (W.T @ A)
    A = np.zeros((HC, 2 * H), np.float32)
    for h in range(H):
        A[h * C:(h + 1) * C, h] = a_left[h]
        A[h * C:(h + 1) * C, H + h] = a_right[h]
    B = (W.T.astype(np.float64) @ A.astype(np.float64)).astype(np.float32)
    wtb = np.zeros((IN, P1COLS), np.float32)
    for h in range(H):
        wtb[:, h * C1:h * C1 + C] = W.T[:, h * C:(h + 1) * C]
    wtb[:, WH1:WH1 + H] = B[:, :H]
    wtb[:, WH1 + H:] = B[:, H:]
    wtb = wtb.astype(BF16)

    core = dst // NPC
    r_src = src % NPC
    c_src = src // NPC
    is_a = r_src < ROWS_A
    gidx = np.where(is_a, c_src * ROWS_A + r_src,
                    c_src * ROWS_B + (r_src - ROWS_A)).astype(np.int64)

    capA = capB = 0
    lists = {}
    for c in range(NC):
        m = core == c
        s_c, d_c, g_c, a_c = src[m], dst[m], gidx[m], is_a[m]
        dl = d_c - c * NPC
        b_c = dl // BUCKET
        order = np.lexsort((s_c, b_c))
        s_c, dl, b_c, g_c, a_c = (s_c[order], dl[order], b_c[order],
                                  g_c[order], a_c[order])
        cnt = np.bincount(b_c, minlength=NBUCK)
        starts = np.concatenate([[0], np.cumsum(cnt)[:-1]])
        for b in range(NBUCK):
            sl = slice(starts[b], starts[b] + cnt[b])
            aa = a_c[sl]
            lists[(c, b)] = (g_c[sl][aa], dl[sl][aa] - b * BUCKET,
                             g_c[sl][~aa], dl[sl][~aa] - b * BUCKET)
            capA = max(capA, (int(aa.sum()) + 127) // 128)
            capB = max(capB, (int((~aa).sum()) + 127) // 128)

    nblkb = capA + capB
    nblk = NBUCK * nblkb
    nblkA = NBUCK * capA
    nblkB = NBUCK * capB
    n_sc = (NBUCK + SC_BUCKETS - 1) // SC_BUCKETS

    idxA = np.zeros((NC, 128, nblkA * 8), np.int16)
    idxB = np.zeros((NC, 128, nblkB * 8), np.int16)
    dloc_u = np.full((NC, 128, nblk), 200.0, BF16)
    dlocT = np.full((NC, 128, nblk * 128), -1, np.int8)
    xT = np.zeros((NC, IN, XT_PAD), BF16)

    for c in range(NC):
        iA = np.zeros(nblkA * 128, np.int64)
        iB = np.zeros(nblkB * 128, np.int64)
        dA = np.full((nblkA, 128), -1, np.int64)
        dB = np.full((nblkB, 128), -1, np.int64)
        for b in range(NBUCK):
            gA, dlA, gB, dlB = lists[(c, b)]
            oa = b * capA * 128
            ob = b * capB * 128
            iA[oa:oa + len(gA)] = gA
            iB[ob:ob + len(gB)] = gB
            fa = dA[b * capA:(b + 1) * capA].reshape(-1)
            fa[:len(dlA)] = dlA
            fb = dB[b * capB:(b + 1) * capB].reshape(-1)
            fb[:len(dlB)] = dlB
        idxA[c] = _wrap16(iA)
        idxB[c] = _wrap16(iB)
        # slot k of block j = (partition k%128); dA rows are flat slot runs
        dA = dA.reshape(nblkA, 128)
        dB = dB.reshape(nblkB, 128)
        # dloc_u: SC-major order [per SC: A-blocks | B-blocks], [128, nblk]
        off = 0
        for sc in range(n_sc):
            b0 = sc * SC_BUCKETS
            bs = range(b0, min(b0 + SC_BUCKETS, NBUCK))
            for b in bs:
                blk = dA[b * capA:(b + 1) * capA]     # [capA, 128]
                v = np.where(blk < 0, 200.0, blk).astype(np.float32)
                dloc_u[c, :, off:off + capA] = v.T.astype(BF16)
                off += capA
            for b in bs:
                blk = dB[b * capB:(b + 1) * capB]
                v = np.where(blk < 0, 200.0, blk).astype(np.float32)
                dloc_u[c, :, off:off + capB] = v.T.astype(BF16)
                off += capB
        # dlocT: BUCKET-major order [per bucket: A-blocks | B-blocks],
        # transposed and replicated: [128(any), (b*nblkb + j)*128 + p]
        for b in range(NBUCK):
            base = b * nblkb * 128
            rows = np.concatenate(
                [dA[b * capA:(b + 1) * capA], dB[b * capB:(b + 1) * capB]],
                axis=0)                                # [nblkb, 128]
            dlocT[c, :, base:base + nblkb * 128] = np.broadcast_to(
                rows.reshape(-1).astype(np.int8), (128, nblkb * 128))

        xs = x[c * NPC:(c + 1) * NPC].astype(BF16)
        xT[c, :, :NPC] = xs.T

    iota = np.tile(np.arange(128, dtype=np.float32)[None, :], (128, 1)).astype(BF16)
    iotaP = np.arange(128, dtype=np.int8).reshape(128, 1)
    iotaPb = np.arange(128, dtype=np.float32).reshape(128, 1).astype(BF16)
    return (wtb, idxA, idxB, dloc_u, dlocT, xT, iota, iotaP, iotaPb,
            capA, capB)


def _build_program(capA, capB):
    nblkb = capA + capB
    nblk = NBUCK * nblkb
    nblkA = NBUCK * capA
    nblkB = NBUCK * capB
    f32 = mybir.dt.float32
    bf16 = mybir.dt.bfloat16
    i16 = mybir.dt.int16
    i8 = mybir.dt.int8

    nc = bass.Bass(trn_type="TRN2", num_devices=NC, num_swdge_queues=NQ)
    xT_in = nc.declare_dram_parameter("xT", [IN, XT_PAD], bf16, isOutput=False)
    wtb_in = nc.declare_dram_parameter("wtb", [IN, P1COLS], bf16, isOutput=False)
    idxA_in = nc.declare_dram_parameter("idxA", [128, nblkA * 8], i16, isOutput=False)
    idxB_in = nc.declare_dram_parameter("idxB", [128, nblkB * 8], i16, isOutput=False)
    dloc_in = nc.declare_dram_parameter("dloc", [128, nblk], bf16, isOutput=False)
    dlocT_in = nc.declare_dram_parameter("dlocT", [128, nblk * 128], i8, isOutput=False)
    iota_in = nc.declare_dram_parameter("iota", [128, 128], bf16, isOutput=False)
    iotaP_in = nc.declare_dram_parameter("iotaP", [128, 1], i8, isOutput=False)
    iotaPb_in = nc.declare_dram_parameter("iotaPb", [128, 1], bf16, isOutput=False)
    out_ext = nc.declare_dram_parameter("out", [NPC, HC], bf16, isOutput=True)

    tbl_locA = nc.dram_tensor("tbl_locA", [ROWS_A, TROW], bf16)
    tbl_locB = nc.dram_tensor("tbl_locB", [ROWS_B, TROW], bf16)
    tblA = nc.dram_tensor("tblA", [NC * ROWS_A, TROW], bf16, addr_space="Shared")
    tblB = nc.dram_tensor("tblB", [NC * ROWS_B, TROW], bf16, addr_space="Shared")
    vs_tbl = nc.dram_tensor("vs_tbl", [128, NBUCK * WH1], bf16)

    with tile.TileContext(nc) as tc:
        with tc.tile_pool(name="cst", bufs=1) as cst:
            # er for all buckets, bucket-partition layout [p, b, h]; filled
            # directly from phase-1 PSUM (dead tail rows produce er=0)
            er_full = cst.tile([128, NBUCK * H], bf16)
            er_full3 = er_full[:].rearrange("p (b h) -> p b h", h=H)
            # ---------------- phase 1: Wh1 / el / er ----------------
            with tc.tile_pool(name="p1w", bufs=1) as p1w, \
                 tc.tile_pool(name="p1", bufs=3) as p1, \
                 tc.tile_pool(name="ps1", bufs=2, space="PSUM") as ps1:
                xts = []
                wtbs = []
                for k in range(2):
                    t = p1w.tile([128, XT_PAD], bf16, tag=f"xt{k}")
                    nc.sync.dma_start(out=t[:], in_=xT_in[k * 128:(k + 1) * 128, :])
                    xts.append(t)
                    u = p1w.tile([128, P1COLS], bf16, tag=f"wtb{k}")
                    nc.sync.dma_start(out=u[:], in_=wtb_in[k * 128:(k + 1) * 128, :])
                    wtbs.append(u)
                groups = []
                for b0s, b1s in ((0, NBUCK_A), (NBUCK_A, NBUCK - 1)):
                    b = b0s
                    while b < b1s:
                        g = min(4, b1s - b)
                        groups.append((b, g))
                        b += g
                groups.append((NBUCK - 1, 1))
                BANK = 512          # f32 elems per PSUM bank
                for b0, g in groups:
                    ps = ps1.tile([128, 4 * BANK], f32)
                    ps3 = ps[:].rearrange("p (g y) -> p g y", y=BANK)
                    for i in range(g):
                        tn = b0 + i
                        for k in range(2):
                            nc.tensor.matmul(
                                out=ps3[:, i, 0:P1COLS],
                                lhsT=xts[k][:, tn * 128:(tn + 1) * 128],
                                rhs=wtbs[k][:],
                                start=(k == 0), stop=(k == 1),
                            )
                    sb = p1.tile([128, 4 * P1COLS], bf16)
                    sb3g = sb[:, 0:g * P1COLS].rearrange("p (g y) -> p g y", y=P1COLS)
                    nc.scalar.activation(out=sb3g,
                                         in_=ps3[:, 0:g, 0:P1COLS],
                                         func=mybir.ActivationFunctionType.Copy)
                    sb4 = sb[:, 0:g * P1COLS].rearrange(
                        "p (g y) -> p g y", y=P1COLS)[:, :, 0:WH1].rearrange(
                        "p g (h c) -> p g h c", c=C1)
                    nc.vector.memset(sb4[:, :, :, C:C1], 1.0)
                    nc.scalar.activation(out=er_full3[:, b0:b0 + g, :],
                                         in_=ps3[:, 0:g, PAY:P1COLS],
                                         func=mybir.ActivationFunctionType.Copy)
                    sb3 = sb[:, 0:g * P1COLS].rearrange("p (g y) -> p g y", y=P1COLS)
                    if b0 + g <= NBUCK_A:
                        dst = tbl_locA[b0 * 128:(b0 + g) * 128, 0:PAY]
                    else:
                        r0 = b0 * 128 - ROWS_A
                        rows = min(g * 128, ROWS_B - r0)
                        dst = tbl_locB[r0:r0 + rows, 0:PAY]
                    if dst.shape[0] == g * 128:
                        dst3 = dst.rearrange("(g p) y -> p g y", p=128)
                        nc.sync.dma_start(out=dst3, in_=sb3[:, :, 0:PAY])
                    else:
                        nc.sync.dma_start(out=dst, in_=sb[:dst.shape[0], 0:PAY])
                    # self-loop weights/messages (local data only): vs rows
                    es = p1.tile([128, 4 * H], f32, tag="es")
                    es3 = es[:, 0:g * H].rearrange("p (g h) -> p g h", h=H)
                    nc.vector.tensor_tensor(
                        out=es3, in0=sb3[:, :, WH1:PAY],
                        in1=er_full3[:, b0:b0 + g, :], op=mybir.AluOpType.add)
                    ws1 = p1.tile([128, 4 * H], bf16, tag="ws1")
                    nc.scalar.activation(out=ws1[:, 0:g * H], in_=es[:, 0:g * H],
                                         func=mybir.ActivationFunctionType.Exp)
                    ws2 = p1.tile([128, 4 * H], bf16, tag="ws2")
                    nc.scalar.activation(out=ws2[:, 0:g * H], in_=es[:, 0:g * H],
                                         scale=NEG,
                                         func=mybir.ActivationFunctionType.Exp)
                    ws = p1.tile([128, 4 * H], bf16, tag="ws")
                    nc.vector.tensor_tensor(out=ws[:, 0:g * H], in0=ws1[:, 0:g * H],
                                            in1=ws2[:, 0:g * H],
                                            op=mybir.AluOpType.max)
                    vs = p1.tile([128, 4 * WH1], bf16, tag="vs")
                    vs4 = vs[:, 0:g * WH1].rearrange("p (g h c) -> p g h c",
                                                     h=H, c=C1)
                    sbw4 = sb3[:, :, 0:WH1].rearrange("p g (h c) -> p g h c", c=C1)
                    ws3 = ws[:, 0:g * H].rearrange("p (g h) -> p g h", h=H)
                    nc.vector.tensor_tensor(
                        out=vs4, in0=sbw4,
                        in1=ws3.to_broadcast([128, g, H, C1]),
                        op=mybir.AluOpType.mult)
                    nc.sync.dma_start(
                        out=vs_tbl[:, b0 * WH1:(b0 + g) * WH1],
                        in_=vs[:, 0:g * WH1])

            # ---------------- all-gather both half tables ----------------
            nc.gpsimd.collective_compute(
                "AllGather", mybir.AluOpType.bypass,
                replica_groups=[list(range(NC))],
                ins=[tbl_locA[:].opt()], outs=[tblA[:].opt()],
            )
            nc.gpsimd.collective_compute(
                "AllGather", mybir.AluOpType.bypass,
                replica_groups=[list(range(NC))],
                ins=[tbl_locB[:].opt()], outs=[tblB[:].opt()],
            )
            nc.gpsimd.load_library(library_config.mlp)

            iota_b = cst.tile([128, 128], bf16)
            nc.sync.dma_start(out=iota_b[:], in_=iota_in[:, :])
            iota_m = iota_b[:].rearrange("p (b n) -> p b n", b=1)
            iotaP_sb = cst.tile([128, 1], i8)
            nc.sync.dma_start(out=iotaP_sb[:], in_=iotaP_in[:, :])
            iotaP_m = iotaP_sb[:].rearrange("p (b n) -> p b n", b=1)
            iotaPb_sb = cst.tile([128, 1], bf16)
            nc.sync.dma_start(out=iotaPb_sb[:], in_=iotaPb_in[:, :])
            ident = cst.tile([128, 128], bf16)
            nc.vector.tensor_tensor(
                out=ident[:], in0=iotaPb_sb[:].to_broadcast([128, 128]),
                in1=iota_b[:], op=mybir.AluOpType.is_equal)

            idxA_sb = cst.tile([128, nblkA * 8], i16)
            nc.sync.dma_start(out=idxA_sb[:], in_=idxA_in[:, :])
            idxB_sb = cst.tile([128, nblkB * 8], i16)
            nc.sync.dma_start(out=idxB_sb[:], in_=idxB_in[:, :])
            dloc_sb = cst.tile([128, nblk], bf16)
            nc.sync.dma_start(out=dloc_sb[:], in_=dloc_in[:, :])
            # ---------------- er expansion on PE (overlaps AllGather) ------
            er_e = cst.tile([128, nblk * 8], bf16)   # SC-major slot order
            with tc.tile_pool(name="otq", bufs=2) as otq, \
                 tc.tile_pool(name="dtq", bufs=2) as dtq, \
                 tc.tile_pool(name="pse", bufs=2, space="PSUM") as pse, \
                 tc.tile_pool(name="gpa", bufs=12) as gpa, \
                 tc.tile_pool(name="gpb", bufs=4) as gpb, \
                 tc.tile_pool(name="vp", bufs=2) as vp, \
                 tc.tile_pool(name="otp", bufs=3) as otp, \
                 tc.tile_pool(name="sp", bufs=2) as sp, \
                 tc.tile_pool(name="sf", bufs=3) as sf, \
                 tc.tile_pool(name="np_", bufs=3) as np_, \
                 tc.tile_pool(name="ps2", bufs=4, space="PSUM") as ps2p:
                n_sc = (NBUCK + SC_BUCKETS - 1) // SC_BUCKETS
                for b in range(NBUCK):
                    dT = dtq.tile([128, nblkb * 128], i8, tag="dT")
                    nc.sync.dma_start(
                        out=dT[:],
                        in_=dlocT_in[:, b * nblkb * 128:(b + 1) * nblkb * 128])
                    dT3 = dT[:].rearrange("p (j n) -> p j n", n=128)
                    OTT = otq.tile([128, nblkb * 128], bf16, tag="OTT")
                    OTT3 = OTT[:].rearrange("p (j n) -> p j n", n=128)
                    nc.vector.tensor_tensor(
                        out=OTT3, in0=iotaP_m.to_broadcast([128, nblkb, 128]),
                        in1=dT3, op=mybir.AluOpType.is_equal)
                    pe = pse.tile([128, nblkb * H], f32)
                    pe3 = pe[:].rearrange("p (j h) -> p j h", h=H)
                    for j in range(nblkb):
                        nc.tensor.matmul(
                            out=pe3[:, j, :], lhsT=OTT3[:, j, :],
                            rhs=er_full3[:, b, :], start=True, stop=True)
                    # copy into er_e at SC-major positions (A-run | B-run)
                    sc = b // SC_BUCKETS
                    b0 = sc * SC_BUCKETS
                    nbk = min(SC_BUCKETS, NBUCK - b0)
                    off_sc = b0 * nblkb
                    offA = (off_sc + (b - b0) * capA) * 8
                    offB = (off_sc + nbk * capA + (b - b0) * capB) * 8
                    nc.scalar.activation(
                        out=er_e[:, offA:offA + capA * 8],
                        in_=pe[:, 0:capA * 8],
                        func=mybir.ActivationFunctionType.Copy)
                    nc.scalar.activation(
                        out=er_e[:, offB:offB + capB * 8],
                        in_=pe[:, capA * 8:nblkb * 8],
                        func=mybir.ActivationFunctionType.Copy)

                # ---------------- phase 2: gather / score / scatter --------
                pend = {}
                qctr = [0]

                def next_q():
                    q = qctr[0] % NQ
                    qctr[0] += 1
                    return q

                def emit_A(sc):
                    b0 = sc * SC_BUCKETS
                    nbk = min(SC_BUCKETS, NBUCK - b0)
                    nbA = nbk * capA
                    nbB = nbk * capB
                    nb = nbA + nbB
                    GA = gpa.tile([128, nbA * GELEM], bf16, tag="GA")
                    GA3 = GA[:].rearrange("p (b y) -> p b y", y=GELEM)
                    a0 = b0 * capA * 8
                    _dma_gather_raw(
                        nc.gpsimd, GA3[:, 0:nbA, :], tblA[:],
                        idxA_sb[:, a0:a0 + nbA * 8], nbA * 128,
                        GELEM, TROW, queue_num=next_q())
                    pend[sc] = (GA3, None, b0, nbk, nbA, nbB, nb)

                def emit_B(sc):
                    GA3, _, b0, nbk, nbA, nbB, nb = pend[sc]
                    GB = gpb.tile([128, nbB * GELEM], bf16, tag="GB")
                    GB3 = GB[:].rearrange("p (b y) -> p b y", y=GELEM)
                    bb0 = b0 * capB * 8
                    _dma_gather_raw(
                        nc.gpsimd, GB3[:, 0:nbB, :], tblB[:],
                        idxB_sb[:, bb0:bb0 + nbB * 8], nbB * 128,
                        GELEM, TROW, queue_num=next_q())
                    pend[sc] = (GA3, GB3, b0, nbk, nbA, nbB, nb)

                def emit_compute(sc):
                    GA3, GB3, b0, nbk, nbA, nbB, nb = pend.pop(sc)
                    off_u = b0 * nblkb
                    # scores: e = el + er ; leaky ; exp
                    e_t = sp.tile([128, nb * H], f32, tag="e")
                    e3 = e_t[:].rearrange("p (b h) -> p b h", h=H)
                    er_sc3 = er_e[:, off_u * 8:(off_u + nb) * 8].rearrange(
                        "p (b h) -> p b h", h=H)
                    nc.vector.tensor_tensor(
                        out=e3[:, 0:nbA, :], in0=GA3[:, :, WH1:PAY],
                        in1=er_sc3[:, 0:nbA, :], op=mybir.AluOpType.add)
                    nc.vector.tensor_tensor(
                        out=e3[:, nbA:nb, :], in0=GB3[:, :, WH1:PAY],
                        in1=er_sc3[:, nbA:nb, :], op=mybir.AluOpType.add)
                    # w = exp(leaky(e)) = max(exp(e), exp(NEG*e)) (exp monotone)
                    w_t = sp.tile([128, nb * H], bf16, tag="w")
                    w1_t = sp.tile([128, nb * H], bf16, tag="w1")
                    nc.scalar.activation(
                        out=w1_t[:], in_=e_t[:],
                        func=mybir.ActivationFunctionType.Exp)
                    w2_t = sp.tile([128, nb * H], bf16, tag="w2")
                    nc.scalar.activation(
                        out=w2_t[:], in_=e_t[:], scale=NEG,
                        func=mybir.ActivationFunctionType.Exp)
                    nc.vector.tensor_tensor(
                        out=w_t[:], in0=w1_t[:], in1=w2_t[:],
                        op=mybir.AluOpType.max)
                    w3 = w_t[:].rearrange("p (b h) -> p b h", h=H)

                    # one-op one-hot build for the whole super-chunk
                    OT = otp.tile([128, nb * 128], bf16, tag="OT")
                    OT3 = OT[:].rearrange("p (b n) -> p b n", n=128)
                    d3 = dloc_sb[:, off_u:off_u + nb].to_broadcast([128, nb, 128])
                    i3 = iota_m.to_broadcast([128, nb, 128])
                    nc.vector.tensor_tensor(out=OT3, in0=d3, in1=i3,
                                            op=mybir.AluOpType.is_equal)

                    # fused V multiply (A-part and B-part)
                    V = vp.tile([128, nb * WH1], bf16, tag="V")
                    V3 = V[:].rearrange("p (b y) -> p b y", y=WH1)
                    V4 = V[:].rearrange("p (b h c) -> p b h c", h=H, c=C1)
                    GA4 = GA3[:, :, 0:WH1].rearrange("p b (h c) -> p b h c", c=C1)
                    GB4 = GB3[:, :, 0:WH1].rearrange("p b (h c) -> p b h c", c=C1)
                    w4 = w3.to_broadcast([128, nb, H, C1])
                    nc.vector.tensor_tensor(out=V4[:, 0:nbA], in0=GA4,
                                            in1=w4[:, 0:nbA],
                                            op=mybir.AluOpType.mult)
                    nc.vector.tensor_tensor(out=V4[:, nbA:nb], in0=GB4,
                                            in1=w4[:, nbA:nb],
                                            op=mybir.AluOpType.mult)

                    # per bucket: scatter matmuls + self-loop fold + normalize
                    for bb in range(nbk):
                        bucket = b0 + bb
                        vsb = sf.tile([128, WH1], bf16, tag="vsb")
                        nc.sync.dma_start(
                            out=vsb[:],
                            in_=vs_tbl[:, bucket * WH1:(bucket + 1) * WH1])

                        blks = ([bb * capA + j for j in range(capA)] +
                                [nbA + bb * capB + j for j in range(capB)])
                        ps = ps2p.tile([128, WH1], f32)
                        for i, blk in enumerate(blks):
                            nc.tensor.matmul(
                                out=ps[:], lhsT=OT3[:, blk, :], rhs=V3[:, blk, :],
                                start=(i == 0), stop=False)
                        nc.tensor.matmul(
                            out=ps[:], lhsT=ident[:], rhs=vsb[:],
                            start=False, stop=True)

                        ps4 = ps[:].rearrange("p (h c) -> p h c", c=C1)
                        # den >= w_self > 0, so EPS=1e-16 is negligible
                        rec = np_.tile([128, H], f32, tag="rec")
                        nc.vector.reciprocal(rec[:], ps4[:, :, C])
                        ot = np_.tile([128, HC], bf16, tag="ot")
                        ot3 = ot[:].rearrange("p (h c) -> p h c", c=C)
                        r3 = rec[:].to_broadcast([128, H, C])
                        nc.vector.tensor_tensor(out=ot3, in0=ps4[:, :, 0:C],
                                                in1=r3, op=mybir.AluOpType.mult)
                        rows = min(128, NPC - bucket * 128)
                        nc.sync.dma_start(
                            out=out_ext[bucket * 128:bucket * 128 + rows, :],
                            in_=ot[:rows, :])

                # A-gathers run LA buckets ahead (through the AG-B window);
                # B-gathers + compute trail together.
                LA = 11
                for t in range(n_sc + LA):
                    if t >= LA:
                        emit_B(t - LA)
                        emit_compute(t - LA)
                    if t < n_sc:
                        emit_A(t)

    _split_excess_waits(nc)
    _move_reload_after_collectives(nc)
    lower_extended_insts(nc)
    return nc


def kernel(**inputs):
    x = np.asarray(inputs["x"], np.float32)
    edge_index = np.asarray(inputs["edge_index"])
    W = np.asarray(inputs["W"], np.float32)
    a_left = np.asarray(inputs["a_left"], np.float32)
    a_right = np.asarray(inputs["a_right"], np.float32)

    (wtb, idxA, idxB, dloc_u, dlocT, xT, iota, iotaP, iotaPb,
     capA, capB) = _host_prep(x, edge_index, W, a_left, a_right)
    nc = _build_program(capA, capB)

    in_maps = []
    for c in range(NC):
        in_maps.append({
            "xT": np.ascontiguousarray(xT[c]),
            "wtb": wtb,
            "idxA": np.ascontiguousarray(idxA[c]),
            "idxB": np.ascontiguousarray(idxB[c]),
            "dloc": np.ascontiguousarray(dloc_u[c]),
            "dlocT": np.ascontiguousarray(dlocT[c]),
            "iota": iota,
            "iotaP": iotaP,
            "iotaPb": iotaPb,
        })

    res = run_bass_kernel_spmd(nc, in_maps, core_ids=list(range(NC)))
    out = np.concatenate([np.asarray(res.results[c]["out"]).astype(np.float32)
                          for c in range(NC)], axis=0)
    return out



# revision 14
# speedup vs baseline: 1.7294x; 1.1222x over previous
"""GAT layer on 8 Trainium2 NeuronCores (Bass/Tile), edge-parallel dst-sharded.

Self-contained. Host preprocesses the graph (dst-shard, bucket sort with
uniform caps, A/B split of each bucket's edges by source half for int16
gather indices; self-loops are NOT materialized as edges). Device program:

  phase 1: per 128-node tile, matmul x @ [W.T | a-folded] producing rows
    [Wh1(264) | el(8) | er(8)]; Wh1 interleaves a constant 1.0 after each
    head's 32 channels so a single multiply by w=exp(leaky(e)) yields both
    the weighted message AND the softmax-denominator column. Rows go to two
    local half-tables (stride 384 for dma_gather's 256B-stride rule) and a
    compact local er table.
  AllGather x2: half tables -> tblA/tblB on every core (AG1 fires once the
    first 25 buckets are written and overlaps the rest of phase 1).
  er expansion on the TENSOR engine (no DMA gather): per bucket, a
    transposed one-hot OTT (built from host-shipped per-slot dst indices)
    times the bucket's er rows gives per-edge-slot er for every block -
    packed into one PSUM tile per bucket and copied out. Runs during the
    AllGather window.
  phase 2 per super-chunk: two dma_gather calls (A/B) fetch table rows by
    src; score ops (add on DVE, leaky+exp on the scalar engine); one-op
    one-hot build; per-block V multiplies; per bucket capA+capB one-hot
    scatter matmuls accumulated in PSUM, then a final identity-matmul adds
    the analytically-computed self-loop contribution, and the bucket is
    normalized by the gathered denominator column and written out.

The SWDGE (GpSimd) descriptor path only carries the unavoidable per-edge
table gather; everything index-like that is bucket-local (er by dst, the
self loops) rides the tensor engine instead.
"""
import sys

for _p in ("/opt/trn_rl_repo",):
    if _p not in sys.path:
        sys.path.insert(0, _p)

import numpy as np
import ml_dtypes

import concourse.bass as bass
import concourse.tile as tile
from concourse import mybir, library_config
from concourse.bass_utils import run_bass_kernel_spmd
from concourse.ap_utils import ap_is_contiguous
from concourse.library_overlay import lower_extended_insts

BF16 = ml_dtypes.bfloat16

N = 50000
E = 800000
IN = 256
H = 8
C = 32
C1 = C + 1            # 33: [Wh_h(32) | 1]
HC = H * C            # 256
WH1 = H * C1          # 264
NC = 8
NPC = N // NC         # 6250 nodes per core
BUCKET = 128
NBUCK = (NPC + BUCKET - 1) // BUCKET   # 49
XT_PAD = NBUCK * 128                   # 6272
PAY = WH1 + H         # 272: gather payload [Wh1(264) | el(8)]
P1COLS = PAY + H      # 280: phase-1 matmul out [Wh1 | el | er]
TROW = 384            # table row stride (256B multiple)
GELEM = 272           # gathered elements per row
NEG = 0.2
EPS = 1e-16
SC_BUCKETS = 1        # buckets per gather super-chunk
NBUCK_A = 25          # buckets 0..24 -> table half A
ROWS_A = NBUCK_A * BUCKET          # 3200
ROWS_B = NPC - ROWS_A              # 3050
USE_PREP_TRIGGER = False
USE_ACT_LRELU = False
NQ = 4

_waitfix_ctr = [0]
_reg_cache = {}


def _split_excess_waits(nc, max_waits=1):
    # walrus in this container caps sync waits per instruction at 1; hoist
    # excess onto same-engine NoOps.
    n_fixed = 0
    for fn in nc.m.functions:
        for bb in fn.blocks:
            insts = bb.instructions
            out = []
            for ins in insts:
                si = ins.sync_info
                waits = list(si.on_wait) if si is not None and si.on_wait else []
                if len(waits) > max_waits:
                    keep = waits[-max_waits:]
                    extra = waits[:-max_waits]
                    for i in range(0, len(extra), max_waits):
                        grp = extra[i:i + max_waits]
                        _waitfix_ctr[0] += 1
                        nop = mybir.InstNoOp(
                            name=f"I-waitfix-{_waitfix_ctr[0]}", ins=[], outs=[])
                        nop.engine = ins.engine
                        nop.sync_info = mybir.SyncInfo(on_wait=grp, on_update=[])
                        nc.register_instruction(nop)
                        out.append(nop)
                    si.on_wait = keep
                    n_fixed += 1
                out.append(ins)
            if len(out) != len(insts):
                bb.instructions = out
    return n_fixed


def _move_reload_after_collectives(nc):
    """The tile scheduler floats the dependency-less library-reload pseudo to
    the top of the program; keep it after the last collective trigger."""
    from concourse import bass_isa
    for fn in nc.m.functions:
        for bb in fn.blocks:
            insts = bb.instructions
            reload_idx = [i for i, ins in enumerate(insts)
                          if isinstance(ins, bass_isa.InstPseudoReloadLibraryIndex)]
            coll_idx = [i for i, ins in enumerate(insts)
                        if isinstance(ins, mybir.InstCollectiveCompute)]
            if not reload_idx or not coll_idx:
                continue
            assert len(reload_idx) == 1
            r = reload_idx[0]
            last_c = max(coll_idx)
            if r > last_c:
                continue
            ins = insts.pop(r)
            insts.insert(last_c, ins)
            bb.instructions = insts
    return nc


def _dma_gather_raw(eng, out_ap, in_ap, idxs_ap, num_idxs, elem_size, elem_step,
                    sem=None, queue_num=0):
    """bass.dma_gather without the elem_size_bytes%256 assert (non-transpose,
    DRAM source, 256B-multiple row stride). sem!=None -> prepare_only."""
    assert idxs_ap.dtype == mybir.dt.int16
    assert in_ap.dtype == out_ap.dtype
    assert ap_is_contiguous(out_ap.ap[1:])
    assert ap_is_contiguous(idxs_ap.ap[1:])
    assert in_ap.ap[0][0] == elem_step
    stride_bytes = elem_step * mybir.dt.size(in_ap.dtype)
    stride_bytes_256 = stride_bytes // 256
    assert stride_bytes_256 * 256 == stride_bytes and stride_bytes_256 < 256
    _in_ap = eng.lower_ap_dma(in_ap, for_custom_bir_dma=True)
    _idxs_ap = eng.lower_ap(idxs_ap)
    _out_ap = eng.lower_ap(out_ap)
    key = (id(eng.bass), num_idxs)
    if key not in _reg_cache:
        _reg_cache[key] = eng.to_reg(num_idxs)
    inst = eng.add_instruction(
        mybir.InstDMAGatherAnt(
            name=eng.bass.get_next_instruction_name(),
            ins=[*_in_ap, _idxs_ap, eng.lower_val_access(_reg_cache[key])],
            outs=[_out_ap],
            transpose=False,
            num_idxs=num_idxs,
            elem_size=elem_size,
            stride_bytes_256=stride_bytes_256,
            gen_mode=int(sem is not None),
            single_packet=False,
            queue_num=queue_num,
            sbuf_tokens_per_rank=0,
            sbuf_free_dim_per_rank=0,
            sbuf_free_dim_pad_per_rank=0,
            sbuf_byte_offset=0,
        )
    )
    if sem is not None:
        inst.then_inc(sem, 16)
        return eng._track_prepare_only(inst, queue_num)
    return inst


def _wrap16(vals):
    """Edge-slot int16 index array -> dma_gather layout [128, n/16]."""
    n = len(vals)
    assert n % 16 == 0
    w = np.asarray(vals, np.int16).reshape(n // 16, 16).T
    return np.tile(w, (8, 1))


def _host_prep(x, edge_index, W, a_left, a_right):
    src = np.asarray(edge_index[0], np.int64)
    dst = np.asarray(edge_index[1], np.int64)

    # LPT node->bucket balancing per core: permute each core's nodes so
    # per-bucket in-degree is near-uniform, shaving the block caps.
    deg = np.bincount(dst, minlength=N)
    perm = np.empty(N, np.int64)      # perm[new_local + core*NPC] = old node id
    inv = np.empty(N, np.int64)       # inv[old node id] = new local row
    for c in range(NC):
        nodes = np.arange(c * NPC, (c + 1) * NPC)
        order = nodes[np.argsort(-deg[nodes], kind="stable")]
        cap_nodes = np.full(NBUCK, BUCKET, np.int64)
        cap_nodes[-1] = NPC - (NBUCK - 1) * BUCKET
        load = np.zeros(NBUCK, np.int64)
        fill = np.zeros(NBUCK, np.int64)
        slot_of = np.empty(NPC, np.int64)
        for n in order:
            avail = np.nonzero(fill < cap_nodes)[0]
            b = avail[np.argmin(load[avail])]
            slot_of[n - c * NPC] = b * BUCKET + fill[b]
            load[b] += deg[n]
            fill[b] += 1
        # new local row r holds old node with slot_of == r
        loc = np.empty(NPC, np.int64)
        loc[slot_of] = np.arange(NPC)
        perm[c * NPC:(c + 1) * NPC] = c * NPC + loc
        inv[c * NPC + loc] = np.arange(NPC)
    src = inv[src] + (src // NPC) * 0 + (np.asarray(edge_index[0], np.int64) // NPC) * NPC
    dst = inv[dst] + (np.asarray(edge_index[1], np.int64) // NPC) * NPC

    # fold attention vectors through W:  [el|er] = x @ (W.T @ A)
    A = np.zeros((HC, 2 * H), np.float32)
    for h in range(H):
        A[h * C:(h + 1) * C, h] = a_left[h]
        A[h * C:(h + 1) * C, H + h] = a_right[h]
    B = (W.T.astype(np.float64) @ A.astype(np.float64)).astype(np.float32)
    wtb = np.zeros((IN, P1COLS), np.float32)
    for h in range(H):
        wtb[:, h * C1:h * C1 + C] = W.T[:, h * C:(h + 1) * C]
    wtb[:, WH1:WH1 + H] = B[:, :H]
    wtb[:, WH1 + H:] = B[:, H:]
    wtb = wtb.astype(BF16)

    core = dst // NPC
    r_src = src % NPC
    c_src = src // NPC
    is_a = r_src < ROWS_A
    gidx = np.where(is_a, c_src * ROWS_A + r_src,
                    c_src * ROWS_B + (r_src - ROWS_A)).astype(np.int64)

    capA = capB = 0
    lists = {}
    for c in range(NC):
        m = core == c
        s_c, d_c, g_c, a_c = src[m], dst[m], gidx[m], is_a[m]
        dl = d_c - c * NPC
        b_c = dl // BUCKET
        order = np.lexsort((s_c, b_c))
        s_c, dl, b_c, g_c, a_c = (s_c[order], dl[order], b_c[order],
                                  g_c[order], a_c[order])
        cnt = np.bincount(b_c, minlength=NBUCK)
        starts = np.concatenate([[0], np.cumsum(cnt)[:-1]])
        for b in range(NBUCK):
            sl = slice(starts[b], starts[b] + cnt[b])
            aa = a_c[sl]
            lists[(c, b)] = (g_c[sl][aa], dl[sl][aa] - b * BUCKET,
                             g_c[sl][~aa], dl[sl][~aa] - b * BUCKET)
            capA = max(capA, (int(aa.sum()) + 127) // 128)
            capB = max(capB, (int((~aa).sum()) + 127) // 128)

    nblkb = capA + capB
    nblk = NBUCK * nblkb
    nblkA = NBUCK * capA
    nblkB = NBUCK * capB
    n_sc = (NBUCK + SC_BUCKETS - 1) // SC_BUCKETS

    idxA = np.zeros((NC, 128, nblkA * 8), np.int16)
    idxB = np.zeros((NC, 128, nblkB * 8), np.int16)
    dloc_u = np.full((NC, 128, nblk), 200.0, BF16)
    dlocT = np.full((NC, 128, nblk * 128), -1, np.int8)
    xT = np.zeros((NC, IN, XT_PAD), BF16)

    for c in range(NC):
        iA = np.zeros(nblkA * 128, np.int64)
        iB = np.zeros(nblkB * 128, np.int64)
        dA = np.full((nblkA, 128), -1, np.int64)
        dB = np.full((nblkB, 128), -1, np.int64)
        for b in range(NBUCK):
            gA, dlA, gB, dlB = lists[(c, b)]
            oa = b * capA * 128
            ob = b * capB * 128
            iA[oa:oa + len(gA)] = gA
            iB[ob:ob + len(gB)] = gB
            fa = dA[b * capA:(b + 1) * capA].reshape(-1)
            fa[:len(dlA)] = dlA
            fb = dB[b * capB:(b + 1) * capB].reshape(-1)
            fb[:len(dlB)] = dlB
        idxA[c] = _wrap16(iA)
        idxB[c] = _wrap16(iB)
        # slot k of block j = (partition k%128); dA rows are flat slot runs
        dA = dA.reshape(nblkA, 128)
        dB = dB.reshape(nblkB, 128)
        # dloc_u: SC-major order [per SC: A-blocks | B-blocks], [128, nblk]
        off = 0
        for sc in range(n_sc):
            b0 = sc * SC_BUCKETS
            bs = range(b0, min(b0 + SC_BUCKETS, NBUCK))
            for b in bs:
                blk = dA[b * capA:(b + 1) * capA]     # [capA, 128]
                v = np.where(blk < 0, 200.0, blk).astype(np.float32)
                dloc_u[c, :, off:off + capA] = v.T.astype(BF16)
                off += capA
            for b in bs:
                blk = dB[b * capB:(b + 1) * capB]
                v = np.where(blk < 0, 200.0, blk).astype(np.float32)
                dloc_u[c, :, off:off + capB] = v.T.astype(BF16)
                off += capB
        # dlocT: BUCKET-major order [per bucket: A-blocks | B-blocks],
        # transposed and replicated: [128(any), (b*nblkb + j)*128 + p]
        for b in range(NBUCK):
            base = b * nblkb * 128
            rows = np.concatenate(
                [dA[b * capA:(b + 1) * capA], dB[b * capB:(b + 1) * capB]],
                axis=0)                                # [nblkb, 128]
            dlocT[c, :, base:base + nblkb * 128] = np.broadcast_to(
                rows.reshape(-1).astype(np.int8), (128, nblkb * 128))

        xs = x[perm[c * NPC:(c + 1) * NPC]].astype(BF16)
        xT[c, :, :NPC] = xs.T

    iota = np.tile(np.arange(128, dtype=np.float32)[None, :], (128, 1)).astype(BF16)
    iotaP = np.arange(128, dtype=np.int8).reshape(128, 1)
    iotaPb = np.arange(128, dtype=np.float32).reshape(128, 1).astype(BF16)
    return (wtb, idxA, idxB, dloc_u, dlocT, xT, iota, iotaP, iotaPb,
            capA, capB, perm)


def _build_program(capA, capB):
    nblkb = capA + capB
    nblk = NBUCK * nblkb
    nblkA = NBUCK * capA
    nblkB = NBUCK * capB
    f32 = mybir.dt.float32
    bf16 = mybir.dt.bfloat16
    i16 = mybir.dt.int16
    i8 = mybir.dt.int8

    nc = bass.Bass(trn_type="TRN2", num_devices=NC, num_swdge_queues=NQ)
    xT_in = nc.declare_dram_parameter("xT", [IN, XT_PAD], bf16, isOutput=False)
    wtb_in = nc.declare_dram_parameter("wtb", [IN, P1COLS], bf16, isOutput=False)
    idxA_in = nc.declare_dram_parameter("idxA", [128, nblkA * 8], i16, isOutput=False)
    idxB_in = nc.declare_dram_parameter("idxB", [128, nblkB * 8], i16, isOutput=False)
    dloc_in = nc.declare_dram_parameter("dloc", [128, nblk], bf16, isOutput=False)
    dlocT_in = nc.declare_dram_parameter("dlocT", [128, nblk * 128], i8, isOutput=False)
    iota_in = nc.declare_dram_parameter("iota", [128, 128], bf16, isOutput=False)
    iotaP_in = nc.declare_dram_parameter("iotaP", [128, 1], i8, isOutput=False)
    iotaPb_in = nc.declare_dram_parameter("iotaPb", [128, 1], bf16, isOutput=False)
    out_ext = nc.declare_dram_parameter("out", [NPC, WH1], bf16, isOutput=True)

    tbl_locA = nc.dram_tensor("tbl_locA", [ROWS_A, TROW], bf16)
    tbl_locB = nc.dram_tensor("tbl_locB", [ROWS_B, TROW], bf16)
    tblA = nc.dram_tensor("tblA", [NC * ROWS_A, TROW], bf16, addr_space="Shared")
    tblB = nc.dram_tensor("tblB", [NC * ROWS_B, TROW], bf16, addr_space="Shared")
    vs_tbl = nc.dram_tensor("vs_tbl", [128, NBUCK * WH1], bf16)

    with tile.TileContext(nc) as tc:
        with tc.tile_pool(name="cst", bufs=1) as cst:
            # er for all buckets, bucket-partition layout [p, b, h]; filled
            # directly from phase-1 PSUM (dead tail rows produce er=0)
            er_full = cst.tile([128, NBUCK * H], bf16)
            er_full3 = er_full[:].rearrange("p (b h) -> p b h", h=H)
            # ---------------- phase 1: Wh1 / el / er ----------------
            with tc.tile_pool(name="p1w", bufs=1) as p1w, \
                 tc.tile_pool(name="p1", bufs=3) as p1, \
                 tc.tile_pool(name="ps1", bufs=2, space="PSUM") as ps1:
                xts = []
                wtbs = []
                for k in range(2):
                    t = p1w.tile([128, XT_PAD], bf16, tag=f"xt{k}")
                    nc.sync.dma_start(out=t[:], in_=xT_in[k * 128:(k + 1) * 128, :])
                    xts.append(t)
                    u = p1w.tile([128, P1COLS], bf16, tag=f"wtb{k}")
                    nc.sync.dma_start(out=u[:], in_=wtb_in[k * 128:(k + 1) * 128, :])
                    wtbs.append(u)
                groups = []
                for b0s, b1s in ((0, NBUCK_A), (NBUCK_A, NBUCK - 1)):
                    b = b0s
                    while b < b1s:
                        g = min(4, b1s - b)
                        groups.append((b, g))
                        b += g
                groups.append((NBUCK - 1, 1))
                BANK = 512          # f32 elems per PSUM bank
                for b0, g in groups:
                    ps = ps1.tile([128, 4 * BANK], f32)
                    ps3 = ps[:].rearrange("p (g y) -> p g y", y=BANK)
                    for i in range(g):
                        tn = b0 + i
                        for k in range(2):
                            nc.tensor.matmul(
                                out=ps3[:, i, 0:P1COLS],
                                lhsT=xts[k][:, tn * 128:(tn + 1) * 128],
                                rhs=wtbs[k][:],
                                start=(k == 0), stop=(k == 1),
                            )
                    sb = p1.tile([128, 4 * P1COLS], bf16)
                    sb3g = sb[:, 0:g * P1COLS].rearrange("p (g y) -> p g y", y=P1COLS)
                    nc.scalar.activation(out=sb3g,
                                         in_=ps3[:, 0:g, 0:P1COLS],
                                         func=mybir.ActivationFunctionType.Copy)
                    sb4 = sb[:, 0:g * P1COLS].rearrange(
                        "p (g y) -> p g y", y=P1COLS)[:, :, 0:WH1].rearrange(
                        "p g (h c) -> p g h c", c=C1)
                    nc.vector.memset(sb4[:, :, :, C:C1], 1.0)
                    nc.scalar.activation(out=er_full3[:, b0:b0 + g, :],
                                         in_=ps3[:, 0:g, PAY:P1COLS],
                                         func=mybir.ActivationFunctionType.Copy)
                    sb3 = sb[:, 0:g * P1COLS].rearrange("p (g y) -> p g y", y=P1COLS)
                    if b0 + g <= NBUCK_A:
                        dst = tbl_locA[b0 * 128:(b0 + g) * 128, 0:PAY]
                    else:
                        r0 = b0 * 128 - ROWS_A
                        rows = min(g * 128, ROWS_B - r0)
                        dst = tbl_locB[r0:r0 + rows, 0:PAY]
                    if dst.shape[0] == g * 128:
                        dst3 = dst.rearrange("(g p) y -> p g y", p=128)
                        nc.sync.dma_start(out=dst3, in_=sb3[:, :, 0:PAY])
                    else:
                        nc.sync.dma_start(out=dst, in_=sb[:dst.shape[0], 0:PAY])
                    # self-loop weights/messages (local data only): vs rows
                    es = p1.tile([128, 4 * H], f32, tag="es")
                    es3 = es[:, 0:g * H].rearrange("p (g h) -> p g h", h=H)
                    nc.vector.tensor_tensor(
                        out=es3, in0=sb3[:, :, WH1:PAY],
                        in1=er_full3[:, b0:b0 + g, :], op=mybir.AluOpType.add)
                    ws1 = p1.tile([128, 4 * H], bf16, tag="ws1")
                    nc.scalar.activation(out=ws1[:, 0:g * H], in_=es[:, 0:g * H],
                                         func=mybir.ActivationFunctionType.Exp)
                    ws2 = p1.tile([128, 4 * H], bf16, tag="ws2")
                    nc.scalar.activation(out=ws2[:, 0:g * H], in_=es[:, 0:g * H],
                                         scale=NEG,
                                         func=mybir.ActivationFunctionType.Exp)
                    ws = p1.tile([128, 4 * H], bf16, tag="ws")
                    nc.vector.tensor_tensor(out=ws[:, 0:g * H], in0=ws1[:, 0:g * H],
                                            in1=ws2[:, 0:g * H],
                                            op=mybir.AluOpType.max)
                    vs = p1.tile([128, 4 * WH1], bf16, tag="vs")
                    vs4 = vs[:, 0:g * WH1].rearrange("p (g h c) -> p g h c",
                                                     h=H, c=C1)
                    sbw4 = sb3[:, :, 0:WH1].rearrange("p g (h c) -> p g h c", c=C1)
                    ws3 = ws[:, 0:g * H].rearrange("p (g h) -> p g h", h=H)
                    nc.vector.tensor_tensor(
                        out=vs4, in0=sbw4,
                        in1=ws3.to_broadcast([128, g, H, C1]),
                        op=mybir.AluOpType.mult)
                    nc.sync.dma_start(
                        out=vs_tbl[:, b0 * WH1:(b0 + g) * WH1],
                        in_=vs[:, 0:g * WH1])

            # ---------------- all-gather both half tables ----------------
            nc.gpsimd.collective_compute(
                "AllGather", mybir.AluOpType.bypass,
                replica_groups=[list(range(NC))],
                ins=[tbl_locA[:].opt()], outs=[tblA[:].opt()],
            )
            nc.gpsimd.collective_compute(
                "AllGather", mybir.AluOpType.bypass,
                replica_groups=[list(range(NC))],
                ins=[tbl_locB[:].opt()], outs=[tblB[:].opt()],
            )
            nc.gpsimd.load_library(library_config.mlp)

            iota_b = cst.tile([128, 128], bf16)
            nc.sync.dma_start(out=iota_b[:], in_=iota_in[:, :])
            iota_m = iota_b[:].rearrange("p (b n) -> p b n", b=1)
            iotaP_sb = cst.tile([128, 1], i8)
            nc.sync.dma_start(out=iotaP_sb[:], in_=iotaP_in[:, :])
            iotaP_m = iotaP_sb[:].rearrange("p (b n) -> p b n", b=1)
            iotaPb_sb = cst.tile([128, 1], bf16)
            nc.sync.dma_start(out=iotaPb_sb[:], in_=iotaPb_in[:, :])
            ident = cst.tile([128, 128], bf16)
            nc.vector.tensor_tensor(
                out=ident[:], in0=iotaPb_sb[:].to_broadcast([128, 128]),
                in1=iota_b[:], op=mybir.AluOpType.is_equal)

            idxA_sb = cst.tile([128, nblkA * 8], i16)
            nc.sync.dma_start(out=idxA_sb[:], in_=idxA_in[:, :])
            idxB_sb = cst.tile([128, nblkB * 8], i16)
            nc.sync.dma_start(out=idxB_sb[:], in_=idxB_in[:, :])
            dloc_sb = cst.tile([128, nblk], bf16)
            nc.sync.dma_start(out=dloc_sb[:], in_=dloc_in[:, :])
            # ---------------- er expansion on PE (overlaps AllGather) ------
            er_e = cst.tile([128, nblk * 8], bf16)   # SC-major slot order
            with tc.tile_pool(name="otq", bufs=2) as otq, \
                 tc.tile_pool(name="dtq", bufs=2) as dtq, \
                 tc.tile_pool(name="pse", bufs=2, space="PSUM") as pse, \
                 tc.tile_pool(name="gpa", bufs=12) as gpa, \
                 tc.tile_pool(name="gpb", bufs=4) as gpb, \
                 tc.tile_pool(name="vp", bufs=2) as vp, \
                 tc.tile_pool(name="otp", bufs=3) as otp, \
                 tc.tile_pool(name="sp", bufs=2) as sp, \
                 tc.tile_pool(name="sf", bufs=3) as sf, \
                 tc.tile_pool(name="np_", bufs=3) as np_, \
                 tc.tile_pool(name="ps2", bufs=4, space="PSUM") as ps2p:
                n_sc = (NBUCK + SC_BUCKETS - 1) // SC_BUCKETS
                for b in range(NBUCK):
                    dT = dtq.tile([128, nblkb * 128], i8, tag="dT")
                    nc.sync.dma_start(
                        out=dT[:],
                        in_=dlocT_in[:, b * nblkb * 128:(b + 1) * nblkb * 128])
                    dT3 = dT[:].rearrange("p (j n) -> p j n", n=128)
                    OTT = otq.tile([128, nblkb * 128], bf16, tag="OTT")
                    OTT3 = OTT[:].rearrange("p (j n) -> p j n", n=128)
                    nc.vector.tensor_tensor(
                        out=OTT3, in0=iotaP_m.to_broadcast([128, nblkb, 128]),
                        in1=dT3, op=mybir.AluOpType.is_equal)
                    pe = pse.tile([128, nblkb * H], f32)
                    pe3 = pe[:].rearrange("p (j h) -> p j h", h=H)
                    for j in range(nblkb):
                        nc.tensor.matmul(
                            out=pe3[:, j, :], lhsT=OTT3[:, j, :],
                            rhs=er_full3[:, b, :], start=True, stop=True)
                    # copy into er_e at SC-major positions (A-run | B-run)
                    sc = b // SC_BUCKETS
                    b0 = sc * SC_BUCKETS
                    nbk = min(SC_BUCKETS, NBUCK - b0)
                    off_sc = b0 * nblkb
                    offA = (off_sc + (b - b0) * capA) * 8
                    offB = (off_sc + nbk * capA + (b - b0) * capB) * 8
                    nc.scalar.activation(
                        out=er_e[:, offA:offA + capA * 8],
                        in_=pe[:, 0:capA * 8],
                        func=mybir.ActivationFunctionType.Copy)
                    nc.scalar.activation(
                        out=er_e[:, offB:offB + capB * 8],
                        in_=pe[:, capA * 8:nblkb * 8],
                        func=mybir.ActivationFunctionType.Copy)

                # ---------------- phase 2: gather / score / scatter --------
                pend = {}
                qctr = [0]

                def next_q():
                    q = qctr[0] % NQ
                    qctr[0] += 1
                    return q

                def emit_A(sc):
                    b0 = sc * SC_BUCKETS
                    nbk = min(SC_BUCKETS, NBUCK - b0)
                    nbA = nbk * capA
                    nbB = nbk * capB
                    nb = nbA + nbB
                    GA = gpa.tile([128, nbA * GELEM], bf16, tag="GA")
                    GA3 = GA[:].rearrange("p (b y) -> p b y", y=GELEM)
                    a0 = b0 * capA * 8
                    _dma_gather_raw(
                        nc.gpsimd, GA3[:, 0:nbA, :], tblA[:],
                        idxA_sb[:, a0:a0 + nbA * 8], nbA * 128,
                        GELEM, TROW, queue_num=next_q())
                    pend[sc] = (GA3, None, b0, nbk, nbA, nbB, nb)

                def emit_B(sc):
                    GA3, _, b0, nbk, nbA, nbB, nb = pend[sc]
                    GB = gpb.tile([128, nbB * GELEM], bf16, tag="GB")
                    GB3 = GB[:].rearrange("p (b y) -> p b y", y=GELEM)
                    bb0 = b0 * capB * 8
                    _dma_gather_raw(
                        nc.gpsimd, GB3[:, 0:nbB, :], tblB[:],
                        idxB_sb[:, bb0:bb0 + nbB * 8], nbB * 128,
                        GELEM, TROW, queue_num=next_q())
                    pend[sc] = (GA3, GB3, b0, nbk, nbA, nbB, nb)

                def emit_compute(sc):
                    GA3, GB3, b0, nbk, nbA, nbB, nb = pend.pop(sc)
                    off_u = b0 * nblkb
                    # scores: e = el + er ; leaky ; exp
                    e_t = sp.tile([128, nb * H], f32, tag="e")
                    e3 = e_t[:].rearrange("p (b h) -> p b h", h=H)
                    er_sc3 = er_e[:, off_u * 8:(off_u + nb) * 8].rearrange(
                        "p (b h) -> p b h", h=H)
                    nc.vector.tensor_tensor(
                        out=e3[:, 0:nbA, :], in0=GA3[:, :, WH1:PAY],
                        in1=er_sc3[:, 0:nbA, :], op=mybir.AluOpType.add)
                    nc.vector.tensor_tensor(
                        out=e3[:, nbA:nb, :], in0=GB3[:, :, WH1:PAY],
                        in1=er_sc3[:, nbA:nb, :], op=mybir.AluOpType.add)
                    # w = exp(leaky(e)) = max(exp(e), exp(NEG*e)) (exp monotone)
                    w_t = sp.tile([128, nb * H], bf16, tag="w")
                    w1_t = sp.tile([128, nb * H], bf16, tag="w1")
                    nc.scalar.activation(
                        out=w1_t[:], in_=e_t[:],
                        func=mybir.ActivationFunctionType.Exp)
                    w2_t = sp.tile([128, nb * H], bf16, tag="w2")
                    nc.scalar.activation(
                        out=w2_t[:], in_=e_t[:], scale=NEG,
                        func=mybir.ActivationFunctionType.Exp)
                    nc.vector.tensor_tensor(
                        out=w_t[:], in0=w1_t[:], in1=w2_t[:],
                        op=mybir.AluOpType.max)
                    w3 = w_t[:].rearrange("p (b h) -> p b h", h=H)

                    # one-op one-hot build for the whole super-chunk
                    OT = otp.tile([128, nb * 128], bf16, tag="OT")
                    OT3 = OT[:].rearrange("p (b n) -> p b n", n=128)
                    d3 = dloc_sb[:, off_u:off_u + nb].to_broadcast([128, nb, 128])
                    i3 = iota_m.to_broadcast([128, nb, 128])
                    nc.vector.tensor_tensor(out=OT3, in0=d3, in1=i3,
                                            op=mybir.AluOpType.is_equal)

                    # fused V multiply (A-part and B-part)
                    V = vp.tile([128, nb * WH1], bf16, tag="V")
                    V3 = V[:].rearrange("p (b y) -> p b y", y=WH1)
                    V4 = V[:].rearrange("p (b h c) -> p b h c", h=H, c=C1)
                    GA4 = GA3[:, :, 0:WH1].rearrange("p b (h c) -> p b h c", c=C1)
                    GB4 = GB3[:, :, 0:WH1].rearrange("p b (h c) -> p b h c", c=C1)
                    w4 = w3.to_broadcast([128, nb, H, C1])
                    nc.vector.tensor_tensor(out=V4[:, 0:nbA], in0=GA4,
                                            in1=w4[:, 0:nbA],
                                            op=mybir.AluOpType.mult)
                    nc.vector.tensor_tensor(out=V4[:, nbA:nb], in0=GB4,
                                            in1=w4[:, nbA:nb],
                                            op=mybir.AluOpType.mult)

                    # per bucket: scatter matmuls + self-loop fold + normalize
                    for bb in range(nbk):
                        bucket = b0 + bb
                        vsb = sf.tile([128, WH1], bf16, tag="vsb")
                        nc.sync.dma_start(
                            out=vsb[:],
                            in_=vs_tbl[:, bucket * WH1:(bucket + 1) * WH1])

                        blks = ([bb * capA + j for j in range(capA)] +
                                [nbA + bb * capB + j for j in range(capB)])
                        ps = ps2p.tile([128, WH1], f32)
                        for i, blk in enumerate(blks):
                            nc.tensor.matmul(
                                out=ps[:], lhsT=OT3[:, blk, :], rhs=V3[:, blk, :],
                                start=(i == 0), stop=False)
                        nc.tensor.matmul(
                            out=ps[:], lhsT=ident[:], rhs=vsb[:],
                            start=False, stop=True)

                        # raw [num | den] row out; host performs the divide
                        ot = np_.tile([128, WH1], bf16, tag="ot")
                        nc.scalar.activation(out=ot[:], in_=ps[:],
                                             func=mybir.ActivationFunctionType.Copy)
                        rows = min(128, NPC - bucket * 128)
                        nc.sync.dma_start(
                            out=out_ext[bucket * 128:bucket * 128 + rows, :],
                            in_=ot[:rows, :])

                # A-gathers run LA buckets ahead (through the AG-B window);
                # B-gathers + compute trail together.
                LA = 11
                for t in range(n_sc + LA):
                    if t >= LA:
                        emit_B(t - LA)
                        emit_compute(t - LA)
                    if t < n_sc:
                        emit_A(t)

    _split_excess_waits(nc)
    _move_reload_after_collectives(nc)
    lower_extended_insts(nc)
    return nc


def kernel(**inputs):
    x = np.asarray(inputs["x"], np.float32)
    edge_index = np.asarray(inputs["edge_index"])
    W = np.asarray(inputs["W"], np.float32)
    a_left = np.asarray(inputs["a_left"], np.float32)
    a_right = np.asarray(inputs["a_right"], np.float32)

    (wtb, idxA, idxB, dloc_u, dlocT, xT, iota, iotaP, iotaPb,
     capA, capB, perm) = _host_prep(x, edge_index, W, a_left, a_right)
    nc = _build_program(capA, capB)

    in_maps = []
    for c in range(NC):
        in_maps.append({
            "xT": np.ascontiguousarray(xT[c]),
            "wtb": wtb,
            "idxA": np.ascontiguousarray(idxA[c]),
            "idxB": np.ascontiguousarray(idxB[c]),
            "dloc": np.ascontiguousarray(dloc_u[c]),
            "dlocT": np.ascontiguousarray(dlocT[c]),
            "iota": iota,
            "iotaP": iotaP,
            "iotaPb": iotaPb,
        })

    res = run_bass_kernel_spmd(nc, in_maps, core_ids=list(range(NC)))
    raw = np.concatenate([np.asarray(res.results[c]["out"]).astype(np.float32)
                          for c in range(NC)], axis=0).reshape(N, H, C1)
    out_p = raw[:, :, 0:C] / raw[:, :, C:C1]
    out = np.empty((N, HC), np.float32)
    out[perm] = out_p.reshape(N, HC)
    return out

